# revision 15
# baseline (speedup 1.0000x reference)
"""Trainium2 Bass kernel for nn_MultiHeadAttention_31542239822105.

Math (faithful to reference, incl. softmax over the QUERY axis):
  q = einsum('bsd,hde->bhse', x, Wq) + bq ; same k, v
  scores = q @ k^T * 1/sqrt(DH)          [B,H,Sq,Sk]
  probs  = softmax(scores, axis=2)       # over q (query axis!)
  ctx    = einsum('bhqk,bhke->bhqe', probs, v)
  out    = ctx.reshape(B,S,D) @ Wo + bo

Sharding: data-parallel over batch, 8 cores x 8 batch items. No collectives.

Per-core layout strategy (all matmul contraction dims land on partitions):
  - x is pre-transposed on the HOST to xT [D, tokens] so no on-chip transposes.
  - Q^T,K^T come out of the projection f-major ([feat, token]) with W as the
    stationary operand; V comes out token-major with xT as stationary.
  - scoresT[k,q] = K^T.T @ Q^T per head -> softmax over q is a FREE-axis
    reduction; exp+sum fused into the PSUM eviction on ScalarE (accum_out).
  - 1/denominator is folded into V rows (cheap: S*DH vs S*S elements).
  - ctx for a HEAD PAIR runs in (128,64) column-tiled PE mode: the two heads'
    matmuls occupy disjoint 64-column strips of the array concurrently
    (tile_position (0,0)/(0,64)), so the pair costs one head's stream time
    and no zero-padded V operand is needed.
  - output projection uses ctxT chunks as stationary -> token-major result,
    direct DMA out. 1/sqrt(DH) folded into Wq/bq on the host.

Global software pipeline (the main perf trick vs the naive phase-serial
form): ScalarE's exp chain is the serial bottleneck of the attention phase
(~830ns per [128,577] tile while the PE needs only ~500ns to produce it), so
the PE stream for batch b's attention is interleaved at ~250ns granularity
with "filler" matmuls: the output projection of batch b-1 and the Q/K/V
projections of batch b+1. The PE then never idles long enough to drop out of
its fast-clock state, and each batch costs max(PE, Scalar) instead of
PE_proj + Scalar_attn.

PSUM budget (8 banks): scores pool 2 bufs x 2 banks, ctx-pair pool 1 x 2,
projection pool 2 x 1 (projections write a 512-wide and a tail piece into
separate single-bank tiles so two chunks can be in flight).
"""

import sys

if "/opt/trn_rl_repo" not in sys.path:
    sys.path.insert(0, "/opt/trn_rl_repo")

import numpy as np
import ml_dtypes

import concourse.bass as bass
import concourse.mybir as mybir
import concourse.tile as tile_mod
from concourse.vector_clock import ScopedClock
from concourse.bass_utils import run_bass_kernel_spmd

# ---------------------------------------------------------------- constants
B, S, D, H = 64, 577, 768, 12
DH = D // H          # 64
NCORES = 8
BC = B // NCORES     # 8 batch items per core
DC = D // 128        # 6 d-chunks
FC = D // 128        # 6 f-chunks per projection matrix
M_QK = 2 * FC        # 12 combined Q+K f-chunks
TT = (S + 127) // 128  # 5 token tiles (128,128,128,128,65)
S0 = 512             # PSUM-bank-sized free-dim split: 577 = 512 + 65
S1 = S - S0

BF16 = mybir.dt.bfloat16
F32 = mybir.dt.float32
nbf = ml_dtypes.bfloat16

_TILE_PATCHED = False
_CUR_NC = [None]


def _patch_tile_drain():
    """The walrus build here rejects >1 sync-wait per instruction
    ("Too many sync wait commands"). Two patches:
    1. post-legalize pass that moves extra waits onto single-wait nops
       inserted just before the offending instruction (same engine);
    2. the final SP Drain (emitted after legalize) gets the same split.
    """
    global _TILE_PATCHED
    if _TILE_PATCHED:
        return
    _TILE_PATCHED = True

    _orig_postorder = tile_mod.postorder_instruction_blocks

    def _split_multi_waits(ordered, nc):
        for bbname, insts in ordered.items():
            out = []
            n_split = 0
            for inst in insts:
                si = inst.sync_info
                if si is not None and len(si.on_wait) > 1:
                    waits = list(si.on_wait)
                    for w in waits[:-1]:
                        nop = mybir.InstNoOp(
                            name=nc.get_next_instruction_name(),
                            ins=[],
                            outs=[],
                            bass_is_fusable=False,
                        )
                        nop.engine = inst.engine
                        nop.sync_info = mybir.SyncInfo(on_wait=[w], on_update=[])
                        nc.register_instruction(nop, overwrite=True)
                        out.append(nop)
                        n_split += 1
                    inst.sync_info = mybir.SyncInfo(
                        on_wait=[waits[-1]], on_update=list(si.on_update)
                    )
                out.append(inst)
            ordered[bbname] = out
        return ordered

    def postorder_and_split(ordered, start_bb, postordered):
        # Runs post-sem-assignment, right before lowering: the only spot
        # where the final per-instruction waits are visible and editable.
        nc = _CUR_NC[0]
        _split_multi_waits(ordered, nc)
        return _orig_postorder(ordered, start_bb, postordered)

    tile_mod.postorder_instruction_blocks = postorder_and_split

    def _drain_and_barrier_split(self, tick_clock, wait_clock):
        nc = self.nc
        drain_inst = nc.sync.drain()
        wait_clock.add_sem_waits(
            drain_inst.ins, ScopedClock({None: tick_clock.global_clock})
        )
        si = drain_inst.ins.sync_info
        waits = list(si.on_wait)
        if len(waits) > 1:
            drain_inst.ins.sync_info = mybir.SyncInfo(
                on_wait=[waits[0]], on_update=list(si.on_update)
            )
            for w in waits[1:]:
                nop = nc.sync.nop(nofuse=True)
                nop.ins.sync_info = mybir.SyncInfo(on_wait=[w], on_update=[])
        nc.all_engine_barrier()
        assert self.sems is not None
        popped = nc._tile_sem_poison_stack.pop()
        assert popped is self._sem_poison
        nc.clear_and_free_semaphores(list(self.sems.allocated().values()))
        nc.all_engine_barrier()

    tile_mod.TileContext._drain_and_barrier = _drain_and_barrier_split


# ---------------------------------------------------------------- builder
def build_bass(bc=BC):
    """Emit the per-core kernel for `bc` batch items. Returns nc."""
    _patch_tile_drain()
    nc = bass.Bass()
    _CUR_NC[0] = nc

    xt_d = nc.declare_dram_parameter("xt", [DC, 128, bc, S], BF16, isOutput=False)
    wqk_d = nc.declare_dram_parameter("wqk", [128, M_QK, DC, 128], BF16, isOutput=False)
    wv_d = nc.declare_dram_parameter("wv", [128, DC, D], BF16, isOutput=False)
    wo_d = nc.declare_dram_parameter("wo", [128, FC, D], BF16, isOutput=False)
    bqk_d = nc.declare_dram_parameter("bqk", [128, M_QK], F32, isOutput=False)
    bvbc_d = nc.declare_dram_parameter("bvbc", [128, D], F32, isOutput=False)
    bobc_d = nc.declare_dram_parameter("bobc", [128, D], F32, isOutput=False)
    out_d = nc.declare_dram_parameter("out", [bc, S, D], F32, isOutput=True)

    AF = mybir.ActivationFunctionType

    with tile_mod.TileContext(nc) as tc:
        with (
            tc.tile_pool(name="singles", bufs=1) as singles,
            tc.tile_pool(name="xt", bufs=2) as xpool,
            tc.tile_pool(name="qk", bufs=2) as qkpool,
            tc.tile_pool(name="ktz", bufs=2) as kzpool,
            tc.tile_pool(name="v", bufs=2) as vpool,
            tc.tile_pool(name="probs", bufs=4) as ppool,
            tc.tile_pool(name="den", bufs=4) as dpool,
            tc.tile_pool(name="rd", bufs=4) as rdpool,
            tc.tile_pool(name="vs", bufs=4) as vspool,
            tc.tile_pool(name="ctx", bufs=2) as cpool,
            tc.tile_pool(name="ot", bufs=3) as opool,
            tc.tile_pool(name="psc", bufs=2, space="PSUM") as psc,
            tc.tile_pool(name="pcx", bufs=1, space="PSUM") as pcx,
            tc.tile_pool(name="ppj", bufs=2, space="PSUM") as ppj,
        ):
            # -------- resident weights / biases (tiles only; DMAs are
            # issued after batch 0's x so the first matmul starts early)
            wqk = singles.tile([128, M_QK, DC, 128], BF16)
            wv = singles.tile([128, DC, D], BF16)
            wo = singles.tile([128, FC, D], BF16)
            bqk = singles.tile([128, M_QK], F32)
            bvbc = singles.tile([128, D], F32)
            bobc = singles.tile([128, D], F32)

            bat = {}

            def start_batch(b):
                xt = xpool.tile([128, DC, S], BF16, tag="xt", name=f"xt{b}")
                for dc in range(DC):
                    nc.sync.dma_start(out=xt[:, dc, :], in_=xt_d[dc, :, b, :])
                bat[b] = dict(
                    xt=xt,
                    qk=qkpool.tile([128, FC, S], BF16, tag="qk", name=f"qk{b}"),
                    ktz=kzpool.tile(
                        [128, FC, 2, S], BF16, tag="ktz", name=f"ktz{b}"
                    ),
                    v=vpool.tile([128, TT, D], BF16, tag="v", name=f"v{b}"),
                    ctxT=cpool.tile([128, FC, S], BF16, tag="ctx", name=f"ctxT{b}"),
                    probs={}, den={}, vs={},
                )

            # ---- projection filler generators (each yield = ~1 PE dc-step)
            def qk_chunk_steps(b, m):
                Bt = bat[b]
                psA = ppj.tile([128, S0], F32, tag="pj", name=f"qA{b}_{m}")
                psB = ppj.tile([128, S1], F32, tag="pj", name=f"qB{b}_{m}")
                for dc in range(DC):
                    st, sp = dc == 0, dc == DC - 1
                    nc.tensor.matmul(
                        psA, lhsT=wqk[:, m, dc, :], rhs=Bt["xt"][:, dc, 0:S0],
                        start=st, stop=sp)
                    nc.tensor.matmul(
                        psB, lhsT=wqk[:, m, dc, :], rhs=Bt["xt"][:, dc, S0:S],
                        start=st, stop=sp)
                    if sp:
                        # In the prologue (b==0) ScalarE is idle and VectorE
                        # is the eviction bottleneck -> split evictions.
                        # In steady state ScalarE is the exp chain -> keep
                        # everything on VectorE.
                        if m < FC:
                            if b == 0:
                                nc.scalar.activation(
                                    Bt["qk"][:, m, 0:S0], psA, AF.Identity,
                                    bias=bqk[:, m : m + 1], scale=1.0)
                                nc.scalar.activation(
                                    Bt["qk"][:, m, S0:S], psB, AF.Identity,
                                    bias=bqk[:, m : m + 1], scale=1.0)
                            else:
                                nc.vector.tensor_scalar_add(
                                    Bt["qk"][:, m, 0:S0], psA, bqk[:, m : m + 1])
                                nc.vector.tensor_scalar_add(
                                    Bt["qk"][:, m, S0:S], psB, bqk[:, m : m + 1])
                        else:
                            mk = m - FC
                            if b < 2:
                                # zero halves persist across pool reuse:
                                # only the first user of each buffer pays.
                                nc.vector.memset(Bt["ktz"][64:128, mk, 0, :], 0.0)
                                nc.vector.memset(Bt["ktz"][0:64, mk, 1, :], 0.0)
                            if b == 0:
                                nc.scalar.activation(
                                    Bt["ktz"][0:64, mk, 0, 0:S0], psA[0:64],
                                    AF.Identity, bias=bqk[0:64, m : m + 1],
                                    scale=1.0)
                                nc.scalar.activation(
                                    Bt["ktz"][0:64, mk, 0, S0:S], psB[0:64],
                                    AF.Identity, bias=bqk[0:64, m : m + 1],
                                    scale=1.0)
                            else:
                                nc.vector.tensor_scalar_add(
                                    Bt["ktz"][0:64, mk, 0, 0:S0], psA[0:64],
                                    bqk[0:64, m : m + 1])
                                nc.vector.tensor_scalar_add(
                                    Bt["ktz"][0:64, mk, 0, S0:S], psB[0:64],
                                    bqk[0:64, m : m + 1])
                            nc.vector.tensor_scalar_add(
                                Bt["ktz"][64:128, mk, 1, 0:S0], psA[64:128],
                                bqk[64:128, m : m + 1])
                            nc.vector.tensor_scalar_add(
                                Bt["ktz"][64:128, mk, 1, S0:S], psB[64:128],
                                bqk[64:128, m : m + 1])
                    yield

            def v_tile_steps(b, tt):
                Bt = bat[b]
                tsz = min(128, S - tt * 128)
                t0 = tt * 128
                psA = ppj.tile([128, 512], F32, tag="pj", name=f"vA{b}_{tt}")
                psB = ppj.tile([128, 256], F32, tag="pj", name=f"vB{b}_{tt}")
                for dc in range(DC):
                    st, sp = dc == 0, dc == DC - 1
                    nc.tensor.matmul(
                        psA[:tsz], lhsT=Bt["xt"][:, dc, t0 : t0 + tsz],
                        rhs=wv[:, dc, 0:512], start=st, stop=sp)
                    nc.tensor.matmul(
                        psB[:tsz], lhsT=Bt["xt"][:, dc, t0 : t0 + tsz],
                        rhs=wv[:, dc, 512:D], start=st, stop=sp)
                    if sp:
                        nc.vector.tensor_add(
                            Bt["v"][:tsz, tt, 0:512], psA[:tsz], bvbc[:tsz, 0:512])
                        nc.vector.tensor_add(
                            Bt["v"][:tsz, tt, 512:D], psB[:tsz], bvbc[:tsz, 512:D])
                    yield

            def o_tile_steps(b, tt):
                Bt = bat[b]
                tsz = min(128, S - tt * 128)
                t0 = tt * 128
                psA = ppj.tile([128, 512], F32, tag="pj", name=f"oA{b}_{tt}")
                psB = ppj.tile([128, 256], F32, tag="pj", name=f"oB{b}_{tt}")
                for fc in range(FC):
                    st, sp = fc == 0, fc == FC - 1
                    nc.tensor.matmul(
                        psA[:tsz], lhsT=Bt["ctxT"][:, fc, t0 : t0 + tsz],
                        rhs=wo[:, fc, 0:512], start=st, stop=sp)
                    nc.tensor.matmul(
                        psB[:tsz], lhsT=Bt["ctxT"][:, fc, t0 : t0 + tsz],
                        rhs=wo[:, fc, 512:D], start=st, stop=sp)
                    if sp:
                        ot = opool.tile([128, D], F32, tag="ot", name=f"ot{b}_{tt}")
                        nc.vector.tensor_add(
                            ot[:tsz, 0:512], psA[:tsz], bobc[:tsz, 0:512])
                        nc.vector.tensor_add(
                            ot[:tsz, 512:D], psB[:tsz], bobc[:tsz, 512:D])
                        nc.sync.dma_start(
                            out=out_d[b, t0 : t0 + tsz, :], in_=ot[:tsz])
                    yield

            def gen_fill(b):
                """Filler steps emitted during attention(b): output
                projection of b-1 and Q/K/V projections of b+1, chunk-
                sequential (the pj psum pool holds one chunk's two pieces)."""
                gens = []
                qgens = []
                if b + 1 < bc:
                    start_batch(b + 1)
                    # pair Q and K chunks so head h's operands finish early
                    order = []
                    for i in range(FC):
                        order += [i, FC + i]
                    qgens = [qk_chunk_steps(b + 1, m) for m in order]
                ogens = [o_tile_steps(b - 1, tt) for tt in range(TT)] if b >= 1 else []
                # interleave at chunk granularity: q, o, q, o, ... then v
                qi, oi = iter(qgens), iter(ogens)
                while True:
                    qn = next(qi, None)
                    on = next(oi, None)
                    if qn is None and on is None:
                        break
                    if qn is not None:
                        gens.append(qn)
                    if on is not None:
                        gens.append(on)
                if b + 1 < bc:
                    gens += [v_tile_steps(b + 1, tt) for tt in range(TT)]
                for g in gens:
                    yield from g

            # ---- attention pieces
            def sc_step(b, h, kc):
                Bt = bat[b]
                m, hh = h // 2, h % 2
                ksz = min(128, S - kc * 128)
                k0 = kc * 128
                ps = psc.tile([128, S], F32, tag="sc", name=f"sc{b}_{h}_{kc}")
                nc.tensor.matmul(
                    ps[:ksz, 0:S0], lhsT=Bt["ktz"][:, m, hh, k0 : k0 + ksz],
                    rhs=Bt["qk"][:, m, 0:S0], start=True, stop=True)
                nc.tensor.matmul(
                    ps[:ksz, S0:S], lhsT=Bt["ktz"][:, m, hh, k0 : k0 + ksz],
                    rhs=Bt["qk"][:, m, S0:S], start=True, stop=True)
                nc.scalar.activation(
                    Bt["probs"][h][:ksz, kc, :], ps[:ksz, 0:S], AF.Exp,
                    accum_out=Bt["den"][h][:ksz, kc : kc + 1])

            def vs_prep(b, h):
                """1/den folded into this head's V columns, zero-padded to a
                full 128-col stationary operand (sub-128 tiling modes drop
                the PE out of its fast clock, so ctx uses full-array
                matmuls with the other head's half zeroed). The pad halves
                persist across pool reuse: 12 heads % 4 bufs = 0, so each
                buffer always carries the same head parity."""
                Bt = bat[b]
                rd = rdpool.tile([128, TT], F32, tag="rd", name=f"rd{b}_{h}")
                nc.vector.reciprocal(rd, Bt["den"][h])
                po = (h % 2) * 64
                vs = vspool.tile([128, TT, 128], BF16, tag="vs", name=f"vs{b}_{h}")
                if b == 0 and h < 4:
                    nc.vector.memset(vs[:, :, 64 - po : 128 - po], 0.0)
                for kc in range(TT):
                    ksz = min(128, S - kc * 128)
                    nc.vector.tensor_scalar_mul(
                        vs[:ksz, kc, po : po + DH],
                        Bt["v"][:ksz, kc, h * DH : (h + 1) * DH],
                        rd[:ksz, kc : kc + 1])
                Bt["vs"][h] = vs

            def ctx_pair(b, h0):
                """ctx for heads (h0, h0+1): both accumulate into one
                [128, S] PSUM tile (head h0 owns partitions 0:64, h0+1 owns
                64:128 via the zero-padded halves of vs)."""
                Bt = bat[b]
                ps = pcx.tile([128, S], F32, tag="cx", name=f"cx{b}_{h0}")
                for hh in (h0, h0 + 1):
                    for kc in range(TT):
                        ksz = min(128, S - kc * 128)
                        st = hh == h0 and kc == 0
                        sp = hh == h0 + 1 and kc == TT - 1
                        nc.tensor.matmul(
                            ps[:, 0:S0], lhsT=Bt["vs"][hh][:ksz, kc, :],
                            rhs=Bt["probs"][hh][:ksz, kc, 0:S0],
                            start=st, stop=sp)
                        nc.tensor.matmul(
                            ps[:, S0:S], lhsT=Bt["vs"][hh][:ksz, kc, :],
                            rhs=Bt["probs"][hh][:ksz, kc, S0:S],
                            start=st, stop=sp)
                nc.vector.tensor_copy(Bt["ctxT"][:, h0 // 2, :], ps[:, 0:S])
                # pair's probs/vs no longer needed; let pools rotate
                for hh in (h0, h0 + 1):
                    Bt["probs"].pop(hh, None)
                    Bt["vs"].pop(hh, None)

            # ---- the schedule
            def emit_fill(fill, n):
                for _ in range(n):
                    try:
                        next(fill)
                    except StopIteration:
                        return

            def slot(b, h, fill):
                Bt = bat[b]
                if h >= 1:
                    vs_prep(b, h - 1)
                probs = ppool.tile([128, TT, S], BF16, tag="probs",
                                   name=f"pr{b}_{h}")
                den = dpool.tile([128, TT], F32, tag="den", name=f"dn{b}_{h}")
                nc.vector.memset(den, 1.0)
                Bt["probs"][h] = probs
                Bt["den"][h] = den
                # paced emission: scores tiles gated by the exp chain two
                # tiles back (scores psum pool bufs=2); fillers keep the PE
                # fed while ScalarE drains.
                if h == 0:
                    # batch boundary: ScalarE is draining the previous
                    # batch's trailing exps. Issue next-batch DMAs + filler
                    # and the carry-over ctx pair BEFORE touching the scores
                    # psum pool so the PE never waits on that drain.
                    emit_fill(fill, 1)
                    if b >= 1:
                        ctx_pair(b - 1, 10)
                    emit_fill(fill, 4)
                    sc_step(b, h, 0)
                    emit_fill(fill, 1); sc_step(b, h, 1)
                    emit_fill(fill, 1); sc_step(b, h, 2)
                    emit_fill(fill, 1); sc_step(b, h, 3)
                    emit_fill(fill, 1); sc_step(b, h, 4)
                    emit_fill(fill, 1)
                elif h % 2 == 1:
                    sc_step(b, h, 0)
                    emit_fill(fill, 2); sc_step(b, h, 1)
                    emit_fill(fill, 2); sc_step(b, h, 2)
                    emit_fill(fill, 3); sc_step(b, h, 3)
                    emit_fill(fill, 3); sc_step(b, h, 4)
                    emit_fill(fill, 4)
                else:
                    sc_step(b, h, 0)
                    emit_fill(fill, 1); sc_step(b, h, 1)
                    emit_fill(fill, 1); sc_step(b, h, 2)
                    ctx_pair(b, h - 2)
                    sc_step(b, h, 3)
                    emit_fill(fill, 1); sc_step(b, h, 4)
                    emit_fill(fill, 2)

            # prologue: batch 0 projections, nothing to overlap with.
            nc.sync.dma_start(out=wqk, in_=wqk_d[:])
            nc.sync.dma_start(out=bqk, in_=bqk_d[:])
            nc.sync.dma_start(out=wv, in_=wv_d[:])
            nc.sync.dma_start(out=bvbc, in_=bvbc_d[:])
            nc.sync.dma_start(out=wo, in_=wo_d[:])
            nc.sync.dma_start(out=bobc, in_=bobc_d[:])
            start_batch(0)
            for g in [qk_chunk_steps(0, m) for mm in range(FC)
                      for m in (mm, FC + mm)]:
                for _ in g:
                    pass
            for tt in range(TT):
                for _ in v_tile_steps(0, tt):
                    pass

            def throttled(gen, k):
                """Yield k times per real filler step so a short filler
                stream spreads over the whole batch (last batch has only
                the previous batch's output projection to offer)."""
                while True:
                    try:
                        next(gen)
                    except StopIteration:
                        return
                    for _ in range(k - 1):
                        yield
                    yield

            # main loop
            for b in range(bc):
                fill = gen_fill(b)
                if b == bc - 1:
                    fill = throttled(fill, 4)
                for h in range(H):
                    slot(b, h, fill)
                vs_prep(b, H - 1)
                # drain any remaining fillers at the batch boundary
                emit_fill(fill, 10**6)

            # epilogue: last pair + output projection of the last batch
            ctx_pair(bc - 1, 10)
            for tt in range(TT):
                for _ in o_tile_steps(bc - 1, tt):
                    pass

    return nc


# ---------------------------------------------------------------- host prep
def _prep_shared(Wq, bq, Wk, bk, Wv, bv, Wo, bo):
    """Build the per-core-identical weight operands."""
    scale = np.float32(1.0 / np.sqrt(DH))
    wqf = (Wq.astype(np.float32) * scale).transpose(1, 0, 2).reshape(D, D)
    wkf = Wk.astype(np.float32).transpose(1, 0, 2).reshape(D, D)
    wvf = Wv.astype(np.float32).transpose(1, 0, 2).reshape(D, D)

    def chunk4(wf):  # [d, f] -> [di, m, dc, fi]
        return wf.reshape(DC, 128, FC, 128).transpose(1, 2, 0, 3)

    wqk = np.concatenate([chunk4(wqf), chunk4(wkf)], axis=1)  # [128, 12, 6, 128]
    wv3 = wvf.reshape(DC, 128, D).transpose(1, 0, 2)          # [128, 6, 768]
    wo3 = Wo.astype(np.float32).reshape(FC, 128, D).transpose(1, 0, 2)

    bqf = (bq.astype(np.float32) * scale).reshape(D)
    bkf = bk.astype(np.float32).reshape(D)
    bqk = np.concatenate(
        [bqf.reshape(FC, 128), bkf.reshape(FC, 128)], axis=0
    ).T.copy()                                                # [128, 12]
    bvbc = np.broadcast_to(bv.astype(np.float32).reshape(D), (128, D)).copy()
    bobc = np.broadcast_to(bo.astype(np.float32).reshape(D), (128, D)).copy()

    return {
        "wqk": np.ascontiguousarray(wqk).astype(nbf),
        "wv": np.ascontiguousarray(wv3).astype(nbf),
        "wo": np.ascontiguousarray(wo3).astype(nbf),
        "bqk": np.ascontiguousarray(bqk),
        "bvbc": bvbc,
        "bobc": bobc,
    }


_NC_CACHE = {}


def kernel(x, Wq, bq, Wk, bk, Wv, bv, Wo, bo):
    x = np.asarray(x, dtype=np.float32)
    shared = _prep_shared(
        np.asarray(Wq), np.asarray(bq), np.asarray(Wk), np.asarray(bk),
        np.asarray(Wv), np.asarray(bv), np.asarray(Wo), np.asarray(bo))

    in_maps = []
    for c in range(NCORES):
        xc = x[c * BC : (c + 1) * BC]                    # [BC, S, D]
        xt = xc.transpose(2, 0, 1)                       # [D, BC, S]
        xt = xt.reshape(DC, 128, BC, S).astype(nbf)
        m = dict(shared)
        m["xt"] = np.ascontiguousarray(xt)
        in_maps.append(m)

    if "nc" not in _NC_CACHE:
        _NC_CACHE["nc"] = build_bass()
    nc = _NC_CACHE["nc"]

    res = run_bass_kernel_spmd(nc, in_maps, core_ids=list(range(NCORES)))
    out = np.concatenate([res.results[c]["out"] for c in range(NCORES)], axis=0)
    return out.astype(np.float32)


if __name__ == "__main__":
    rng = np.random.default_rng(0)
    ins = {
        "x": rng.standard_normal((B, S, D), dtype=np.float32),
        "Wq": rng.standard_normal((H, D, DH), dtype=np.float32) * 0.02,
        "bq": np.zeros((H, DH), np.float32),
        "Wk": rng.standard_normal((H, D, DH), dtype=np.float32) * 0.02,
        "bk": np.zeros((H, DH), np.float32),
        "Wv": rng.standard_normal((H, D, DH), dtype=np.float32) * 0.02,
        "bv": np.zeros((H, DH), np.float32),
        "Wo": rng.standard_normal((D, D), dtype=np.float32) * 0.02,
        "bo": np.zeros((D,), np.float32),
    }
    o = kernel(**ins)
    print("out", o.shape, o.dtype, float(np.abs(o).max()))


# revision 22
# speedup vs baseline: 1.0178x; 1.0178x over previous
"""Trainium2 Bass kernel for nn_MultiHeadAttention_31542239822105.

Math (faithful to reference, incl. softmax over the QUERY axis):
  q = einsum('bsd,hde->bhse', x, Wq) + bq ; same k, v
  scores = q @ k^T * 1/sqrt(DH)          [B,H,Sq,Sk]
  probs  = softmax(scores, axis=2)       # over q (query axis!)
  ctx    = einsum('bhqk,bhke->bhqe', probs, v)
  out    = ctx.reshape(B,S,D) @ Wo + bo

Sharding: data-parallel over batch, 8 cores x 8 batch items. No collectives.

Per-core layout strategy (all matmul contraction dims land on partitions):
  - x is pre-transposed on the HOST to xT [D, tokens] so no on-chip transposes.
  - Q^T,K^T come out of the projection f-major ([feat, token]) with W as the
    stationary operand; V comes out token-major with xT as stationary.
  - scoresT[k,q] = K^T.T @ Q^T per head -> softmax over q is a FREE-axis
    reduction; exp+sum fused into the PSUM eviction on ScalarE (accum_out).
  - 1/denominator is folded into V rows (cheap: S*DH vs S*S elements).
  - ctx for a HEAD PAIR runs in (128,64) column-tiled PE mode: the two heads'
    matmuls occupy disjoint 64-column strips of the array concurrently
    (tile_position (0,0)/(0,64)), so the pair costs one head's stream time
    and no zero-padded V operand is needed.
  - output projection uses ctxT chunks as stationary -> token-major result,
    direct DMA out. 1/sqrt(DH) folded into Wq/bq on the host.

Global software pipeline (the main perf trick vs the naive phase-serial
form): ScalarE's exp chain is the serial bottleneck of the attention phase
(~830ns per [128,577] tile while the PE needs only ~500ns to produce it), so
the PE stream for batch b's attention is interleaved at ~250ns granularity
with "filler" matmuls: the output projection of batch b-1 and the Q/K/V
projections of batch b+1. The PE then never idles long enough to drop out of
its fast-clock state, and each batch costs max(PE, Scalar) instead of
PE_proj + Scalar_attn.

PSUM budget (8 banks): scores pool 2 bufs x 2 banks, ctx-pair pool 1 x 2,
projection pool 2 x 1 (projections write a 512-wide and a tail piece into
separate single-bank tiles so two chunks can be in flight).
"""

import sys

if "/opt/trn_rl_repo" not in sys.path:
    sys.path.insert(0, "/opt/trn_rl_repo")

import numpy as np
import ml_dtypes

import concourse.bass as bass
import concourse.mybir as mybir
import concourse.tile as tile_mod
from concourse.vector_clock import ScopedClock
from concourse.bass_utils import run_bass_kernel_spmd

# ---------------------------------------------------------------- constants
B, S, D, H = 64, 577, 768, 12
DH = D // H          # 64
NCORES = 8
BC = B // NCORES     # 8 batch items per core
DC = D // 128        # 6 d-chunks
FC = D // 128        # 6 f-chunks per projection matrix
M_QK = 2 * FC        # 12 combined Q+K f-chunks
TT = (S + 127) // 128  # 5 token tiles (128,128,128,128,65)
S0 = 512             # PSUM-bank-sized free-dim split: 577 = 512 + 65
S1 = S - S0

BF16 = mybir.dt.bfloat16
F32 = mybir.dt.float32
nbf = ml_dtypes.bfloat16

_TILE_PATCHED = False
_CUR_NC = [None]


def _patch_tile_drain():
    """The walrus build here rejects >1 sync-wait per instruction
    ("Too many sync wait commands"). Two patches:
    1. post-legalize pass that moves extra waits onto single-wait nops
       inserted just before the offending instruction (same engine);
    2. the final SP Drain (emitted after legalize) gets the same split.
    """
    global _TILE_PATCHED
    if _TILE_PATCHED:
        return
    _TILE_PATCHED = True

    _orig_postorder = tile_mod.postorder_instruction_blocks

    def _split_multi_waits(ordered, nc):
        for bbname, insts in ordered.items():
            out = []
            n_split = 0
            for inst in insts:
                si = inst.sync_info
                if si is not None and len(si.on_wait) > 1:
                    waits = list(si.on_wait)
                    for w in waits[:-1]:
                        nop = mybir.InstNoOp(
                            name=nc.get_next_instruction_name(),
                            ins=[],
                            outs=[],
                            bass_is_fusable=False,
                        )
                        nop.engine = inst.engine
                        nop.sync_info = mybir.SyncInfo(on_wait=[w], on_update=[])
                        nc.register_instruction(nop, overwrite=True)
                        out.append(nop)
                        n_split += 1
                    inst.sync_info = mybir.SyncInfo(
                        on_wait=[waits[-1]], on_update=list(si.on_update)
                    )
                out.append(inst)
            ordered[bbname] = out
        return ordered

    def postorder_and_split(ordered, start_bb, postordered):
        # Runs post-sem-assignment, right before lowering: the only spot
        # where the final per-instruction waits are visible and editable.
        nc = _CUR_NC[0]
        _split_multi_waits(ordered, nc)
        return _orig_postorder(ordered, start_bb, postordered)

    tile_mod.postorder_instruction_blocks = postorder_and_split

    def _drain_and_barrier_split(self, tick_clock, wait_clock):
        nc = self.nc
        drain_inst = nc.sync.drain()
        wait_clock.add_sem_waits(
            drain_inst.ins, ScopedClock({None: tick_clock.global_clock})
        )
        si = drain_inst.ins.sync_info
        waits = list(si.on_wait)
        if len(waits) > 1:
            drain_inst.ins.sync_info = mybir.SyncInfo(
                on_wait=[waits[0]], on_update=list(si.on_update)
            )
            for w in waits[1:]:
                nop = nc.sync.nop(nofuse=True)
                nop.ins.sync_info = mybir.SyncInfo(on_wait=[w], on_update=[])
        nc.all_engine_barrier()
        assert self.sems is not None
        popped = nc._tile_sem_poison_stack.pop()
        assert popped is self._sem_poison
        nc.clear_and_free_semaphores(list(self.sems.allocated().values()))
        nc.all_engine_barrier()

    tile_mod.TileContext._drain_and_barrier = _drain_and_barrier_split


# ---------------------------------------------------------------- builder
def build_bass(bc=BC):
    """Emit the per-core kernel for `bc` batch items. Returns nc."""
    _patch_tile_drain()
    nc = bass.Bass()
    _CUR_NC[0] = nc

    xt_d = nc.declare_dram_parameter("xt", [DC, 128, bc, S], BF16, isOutput=False)
    wqk_d = nc.declare_dram_parameter("wqk", [128, M_QK, DC, 128], BF16, isOutput=False)
    wv_d = nc.declare_dram_parameter("wv", [128, DC, D], BF16, isOutput=False)
    wo_d = nc.declare_dram_parameter("wo", [128, FC, D], BF16, isOutput=False)
    bqk_d = nc.declare_dram_parameter("bqk", [128, M_QK], F32, isOutput=False)
    bvbc_d = nc.declare_dram_parameter("bvbc", [128, D], F32, isOutput=False)
    bobc_d = nc.declare_dram_parameter("bobc", [128, D], F32, isOutput=False)
    out_d = nc.declare_dram_parameter("out", [bc, S, D], F32, isOutput=True)

    AF = mybir.ActivationFunctionType

    with tile_mod.TileContext(nc) as tc:
        with (
            tc.tile_pool(name="singles", bufs=1) as singles,
            tc.tile_pool(name="xt", bufs=2) as xpool,
            tc.tile_pool(name="qk", bufs=2) as qkpool,
            tc.tile_pool(name="ktz", bufs=2) as kzpool,
            tc.tile_pool(name="v", bufs=2) as vpool,
            tc.tile_pool(name="probs", bufs=4) as ppool,
            tc.tile_pool(name="den", bufs=4) as dpool,
            tc.tile_pool(name="rd", bufs=4) as rdpool,
            tc.tile_pool(name="vs", bufs=4) as vspool,
            tc.tile_pool(name="ctx", bufs=2) as cpool,
            tc.tile_pool(name="ot", bufs=3) as opool,
            tc.tile_pool(name="psc", bufs=2, space="PSUM") as psc,
            tc.tile_pool(name="pcx", bufs=1, space="PSUM") as pcx,
            tc.tile_pool(name="ppj", bufs=2, space="PSUM") as ppj,
        ):
            # -------- resident weights / biases (tiles only; DMAs are
            # issued after batch 0's x so the first matmul starts early)
            wqk = singles.tile([128, M_QK, DC, 128], BF16)
            wv = singles.tile([128, DC, D], BF16)
            wo = singles.tile([128, FC, D], BF16)
            bqk = singles.tile([128, M_QK], F32)
            bvbc = singles.tile([128, D], F32)
            bobc = singles.tile([128, D], F32)

            bat = {}

            def start_batch(b):
                xt = xpool.tile([128, DC, S], BF16, tag="xt", name=f"xt{b}")
                for dc in range(DC):
                    nc.sync.dma_start(out=xt[:, dc, :], in_=xt_d[dc, :, b, :])
                bat[b] = dict(
                    xt=xt,
                    qk=qkpool.tile([128, FC, S], BF16, tag="qk", name=f"qk{b}"),
                    ktz=kzpool.tile(
                        [128, FC, 2, S], BF16, tag="ktz", name=f"ktz{b}"
                    ),
                    v=vpool.tile([128, TT, D], BF16, tag="v", name=f"v{b}"),
                    ctxT=cpool.tile([128, FC, S], BF16, tag="ctx", name=f"ctxT{b}"),
                    probs={}, den={}, vs={},
                )

            # ---- projection filler generators (each yield = ~1 PE dc-step)
            def qk_chunk_steps(b, m, alt=False):
                # alt=True (prologue/epilogue only): use the idle scores
                # pool so twice as many chunks can be in flight.
                Bt = bat[b]
                if alt:
                    ps = psc.tile([128, S], F32, tag="sc", name=f"qP{b}_{m}")
                    psA, psB = ps[:, 0:S0], ps[:, S0:S]
                else:
                    psA = ppj.tile([128, S0], F32, tag="pj", name=f"qA{b}_{m}")
                    psB = ppj.tile([128, S1], F32, tag="pj", name=f"qB{b}_{m}")
                for dc in range(DC):
                    st, sp = dc == 0, dc == DC - 1
                    nc.tensor.matmul(
                        psA, lhsT=wqk[:, m, dc, :], rhs=Bt["xt"][:, dc, 0:S0],
                        start=st, stop=sp)
                    nc.tensor.matmul(
                        psB, lhsT=wqk[:, m, dc, :], rhs=Bt["xt"][:, dc, S0:S],
                        start=st, stop=sp)
                    if sp:
                        # In the prologue (b==0) ScalarE is idle and VectorE
                        # is the eviction bottleneck -> split evictions.
                        # In steady state ScalarE is the exp chain -> keep
                        # everything on VectorE.
                        if m < FC:
                            if b == 0:
                                nc.scalar.activation(
                                    Bt["qk"][:, m, 0:S0], psA, AF.Identity,
                                    bias=bqk[:, m : m + 1], scale=1.0)
                                nc.scalar.activation(
                                    Bt["qk"][:, m, S0:S], psB, AF.Identity,
                                    bias=bqk[:, m : m + 1], scale=1.0)
                            else:
                                nc.vector.tensor_scalar_add(
                                    Bt["qk"][:, m, 0:S0], psA, bqk[:, m : m + 1])
                                nc.vector.tensor_scalar_add(
                                    Bt["qk"][:, m, S0:S], psB, bqk[:, m : m + 1])
                        else:
                            mk = m - FC
                            if b < 2:
                                # zero halves persist across pool reuse:
                                # only the first user of each buffer pays.
                                nc.vector.memset(Bt["ktz"][64:128, mk, 0, :], 0.0)
                                nc.vector.memset(Bt["ktz"][0:64, mk, 1, :], 0.0)
                            if b == 0:
                                nc.scalar.activation(
                                    Bt["ktz"][0:64, mk, 0, 0:S0], psA[0:64],
                                    AF.Identity, bias=bqk[0:64, m : m + 1],
                                    scale=1.0)
                                nc.scalar.activation(
                                    Bt["ktz"][0:64, mk, 0, S0:S], psB[0:64],
                                    AF.Identity, bias=bqk[0:64, m : m + 1],
                                    scale=1.0)
                            else:
                                nc.vector.tensor_scalar_add(
                                    Bt["ktz"][0:64, mk, 0, 0:S0], psA[0:64],
                                    bqk[0:64, m : m + 1])
                                nc.vector.tensor_scalar_add(
                                    Bt["ktz"][0:64, mk, 0, S0:S], psB[0:64],
                                    bqk[0:64, m : m + 1])
                            nc.vector.tensor_scalar_add(
                                Bt["ktz"][64:128, mk, 1, 0:S0], psA[64:128],
                                bqk[64:128, m : m + 1])
                            nc.vector.tensor_scalar_add(
                                Bt["ktz"][64:128, mk, 1, S0:S], psB[64:128],
                                bqk[64:128, m : m + 1])
                    yield

            def v_tile_steps(b, tt, alt=False):
                Bt = bat[b]
                tsz = min(128, S - tt * 128)
                t0 = tt * 128
                if alt:
                    ps = pcx.tile([128, D], F32, tag="cx", name=f"vP{b}_{tt}")
                    psA, psB = ps[:, 0:512], ps[:, 512:D]
                else:
                    psA = ppj.tile([128, 512], F32, tag="pj", name=f"vA{b}_{tt}")
                    psB = ppj.tile([128, 256], F32, tag="pj", name=f"vB{b}_{tt}")
                for dc in range(DC):
                    st, sp = dc == 0, dc == DC - 1
                    nc.tensor.matmul(
                        psA[:tsz], lhsT=Bt["xt"][:, dc, t0 : t0 + tsz],
                        rhs=wv[:, dc, 0:512], start=st, stop=sp)
                    nc.tensor.matmul(
                        psB[:tsz], lhsT=Bt["xt"][:, dc, t0 : t0 + tsz],
                        rhs=wv[:, dc, 512:D], start=st, stop=sp)
                    if sp:
                        nc.vector.tensor_add(
                            Bt["v"][:tsz, tt, 0:512], psA[:tsz], bvbc[:tsz, 0:512])
                        nc.vector.tensor_add(
                            Bt["v"][:tsz, tt, 512:D], psB[:tsz], bvbc[:tsz, 512:D])
                    yield

            def o_tile_steps(b, tt, alt=False):
                Bt = bat[b]
                tsz = min(128, S - tt * 128)
                t0 = tt * 128
                if alt:
                    ps = pcx.tile([128, D], F32, tag="cx", name=f"oP{b}_{tt}")
                    psA, psB = ps[:, 0:512], ps[:, 512:D]
                else:
                    psA = ppj.tile([128, 512], F32, tag="pj", name=f"oA{b}_{tt}")
                    psB = ppj.tile([128, 256], F32, tag="pj", name=f"oB{b}_{tt}")
                for fc in range(FC):
                    st, sp = fc == 0, fc == FC - 1
                    nc.tensor.matmul(
                        psA[:tsz], lhsT=Bt["ctxT"][:, fc, t0 : t0 + tsz],
                        rhs=wo[:, fc, 0:512], start=st, stop=sp)
                    nc.tensor.matmul(
                        psB[:tsz], lhsT=Bt["ctxT"][:, fc, t0 : t0 + tsz],
                        rhs=wo[:, fc, 512:D], start=st, stop=sp)
                    if sp:
                        ot = opool.tile([128, D], F32, tag="ot", name=f"ot{b}_{tt}")
                        nc.vector.tensor_add(
                            ot[:tsz, 0:512], psA[:tsz], bobc[:tsz, 0:512])
                        nc.vector.tensor_add(
                            ot[:tsz, 512:D], psB[:tsz], bobc[:tsz, 512:D])
                        nc.sync.dma_start(
                            out=out_d[b, t0 : t0 + tsz, :], in_=ot[:tsz])
                    yield

            def gen_fill(b):
                """Filler steps emitted during attention(b): output
                projection of b-1 and Q/K/V projections of b+1, chunk-
                sequential (the pj psum pool holds one chunk's two pieces)."""
                gens = []
                qgens = []
                if b + 1 < bc:
                    start_batch(b + 1)
                    # pair Q and K chunks so head h's operands finish early
                    order = []
                    for i in range(FC):
                        order += [i, FC + i]
                    qgens = [qk_chunk_steps(b + 1, m) for m in order]
                ogens = [o_tile_steps(b - 1, tt) for tt in range(TT)] if b >= 1 else []
                # interleave at chunk granularity: q, o, q, o, ... then v
                qi, oi = iter(qgens), iter(ogens)
                while True:
                    qn = next(qi, None)
                    on = next(oi, None)
                    if qn is None and on is None:
                        break
                    if qn is not None:
                        gens.append(qn)
                    if on is not None:
                        gens.append(on)
                if b + 1 < bc:
                    gens += [v_tile_steps(b + 1, tt) for tt in range(TT)]
                for g in gens:
                    yield from g

            # ---- attention pieces
            def sc_step(b, h, kc):
                Bt = bat[b]
                m, hh = h // 2, h % 2
                ksz = min(128, S - kc * 128)
                k0 = kc * 128
                ps = psc.tile([128, S], F32, tag="sc", name=f"sc{b}_{h}_{kc}")
                nc.tensor.matmul(
                    ps[:ksz, 0:S0], lhsT=Bt["ktz"][:, m, hh, k0 : k0 + ksz],
                    rhs=Bt["qk"][:, m, 0:S0], start=True, stop=True)
                nc.tensor.matmul(
                    ps[:ksz, S0:S], lhsT=Bt["ktz"][:, m, hh, k0 : k0 + ksz],
                    rhs=Bt["qk"][:, m, S0:S], start=True, stop=True)
                nc.scalar.activation(
                    Bt["probs"][h][:ksz, kc, :], ps[:ksz, 0:S], AF.Exp,
                    accum_out=Bt["den"][h][:ksz, kc : kc + 1])

            def vs_prep(b, h):
                """1/den folded into this head's V columns, zero-padded to a
                full 128-col stationary operand (sub-128 tiling modes drop
                the PE out of its fast clock, so ctx uses full-array
                matmuls with the other head's half zeroed). The pad halves
                persist across pool reuse: 12 heads % 4 bufs = 0, so each
                buffer always carries the same head parity."""
                Bt = bat[b]
                rd = rdpool.tile([128, TT], F32, tag="rd", name=f"rd{b}_{h}")
                nc.vector.reciprocal(rd, Bt["den"][h])
                po = (h % 2) * 64
                vs = vspool.tile([128, TT, 128], BF16, tag="vs", name=f"vs{b}_{h}")
                if b == 0 and h < 4:
                    nc.vector.memset(vs[:, :, 64 - po : 128 - po], 0.0)
                for kc in range(TT):
                    ksz = min(128, S - kc * 128)
                    nc.vector.tensor_scalar_mul(
                        vs[:ksz, kc, po : po + DH],
                        Bt["v"][:ksz, kc, h * DH : (h + 1) * DH],
                        rd[:ksz, kc : kc + 1])
                Bt["vs"][h] = vs

            def ctx_pair(b, h0, fill=None):
                """ctx for heads (h0, h0+1): both accumulate into one
                [128, S] PSUM tile (head h0 owns partitions 0:64, h0+1 owns
                64:128 via the zero-padded halves of vs). At a batch
                boundary head h0+1's probs are still draining from ScalarE,
                so the caller passes `fill` to pad between the halves."""
                Bt = bat[b]
                ps = pcx.tile([128, S], F32, tag="cx", name=f"cx{b}_{h0}")
                for hh in (h0, h0 + 1):
                    for kc in range(TT):
                        ksz = min(128, S - kc * 128)
                        st = hh == h0 and kc == 0
                        sp = hh == h0 + 1 and kc == TT - 1
                        nc.tensor.matmul(
                            ps[:, 0:S0], lhsT=Bt["vs"][hh][:ksz, kc, :],
                            rhs=Bt["probs"][hh][:ksz, kc, 0:S0],
                            start=st, stop=sp)
                        nc.tensor.matmul(
                            ps[:, S0:S], lhsT=Bt["vs"][hh][:ksz, kc, :],
                            rhs=Bt["probs"][hh][:ksz, kc, S0:S],
                            start=st, stop=sp)
                    if hh == h0 and fill is not None:
                        emit_fill(fill, 3)
                nc.vector.tensor_copy(Bt["ctxT"][:, h0 // 2, :], ps[:, 0:S])
                # pair's probs/vs no longer needed; let pools rotate
                for hh in (h0, h0 + 1):
                    Bt["probs"].pop(hh, None)
                    Bt["vs"].pop(hh, None)

            # ---- the schedule
            def emit_fill(fill, n):
                for _ in range(n):
                    try:
                        next(fill)
                    except StopIteration:
                        return

            def slot(b, h, fill):
                Bt = bat[b]
                if h >= 1:
                    vs_prep(b, h - 1)
                probs = ppool.tile([128, TT, S], BF16, tag="probs",
                                   name=f"pr{b}_{h}")
                den = dpool.tile([128, TT], F32, tag="den", name=f"dn{b}_{h}")
                nc.vector.memset(den, 1.0)
                Bt["probs"][h] = probs
                Bt["den"][h] = den
                # paced emission: scores tiles gated by the exp chain two
                # tiles back (scores psum pool bufs=2); fillers keep the PE
                # fed while ScalarE drains.
                if h == 0:
                    # batch boundary: ScalarE is draining the previous
                    # batch's trailing exps. Issue next-batch DMAs + filler
                    # and the carry-over ctx pair BEFORE touching the scores
                    # psum pool so the PE never waits on that drain.
                    emit_fill(fill, 1)
                    if b >= 1:
                        ctx_pair(b - 1, 10, fill)
                    emit_fill(fill, 1)
                    sc_step(b, h, 0)
                    emit_fill(fill, 1); sc_step(b, h, 1)
                    emit_fill(fill, 1); sc_step(b, h, 2)
                    emit_fill(fill, 1); sc_step(b, h, 3)
                    emit_fill(fill, 1); sc_step(b, h, 4)
                    emit_fill(fill, 1)
                elif h % 2 == 1:
                    sc_step(b, h, 0)
                    emit_fill(fill, 2); sc_step(b, h, 1)
                    emit_fill(fill, 2); sc_step(b, h, 2)
                    emit_fill(fill, 3); sc_step(b, h, 3)
                    emit_fill(fill, 3); sc_step(b, h, 4)
                    emit_fill(fill, 5)
                else:
                    sc_step(b, h, 0)
                    emit_fill(fill, 1); sc_step(b, h, 1)
                    emit_fill(fill, 1); sc_step(b, h, 2)
                    ctx_pair(b, h - 2)
                    sc_step(b, h, 3)
                    emit_fill(fill, 2); sc_step(b, h, 4)
                    emit_fill(fill, 2)

            # prologue: batch 0 projections, nothing to overlap with.
            nc.sync.dma_start(out=wqk, in_=wqk_d[:])
            nc.sync.dma_start(out=bqk, in_=bqk_d[:])
            nc.sync.dma_start(out=wv, in_=wv_d[:])
            nc.sync.dma_start(out=bvbc, in_=bvbc_d[:])
            nc.sync.dma_start(out=wo, in_=wo_d[:])
            nc.sync.dma_start(out=bobc, in_=bobc_d[:])
            start_batch(0)
            order0 = []
            for i in range(FC):
                order0 += [i, FC + i]
            for idx, m in enumerate(order0):
                for _ in qk_chunk_steps(0, m, alt=(idx % 2 == 0)):
                    pass
            for tt in range(TT):
                for _ in v_tile_steps(0, tt, alt=(tt % 2 == 0)):
                    pass

            def throttled(gen, k):
                """Yield k times per real filler step so a short filler
                stream spreads over the whole batch (last batch has only
                the previous batch's output projection to offer)."""
                while True:
                    try:
                        next(gen)
                    except StopIteration:
                        return
                    for _ in range(k - 1):
                        yield
                    yield

            # main loop
            for b in range(bc):
                fill = gen_fill(b)
                if b == bc - 1:
                    fill = throttled(fill, 4)
                for h in range(H):
                    slot(b, h, fill)
                vs_prep(b, H - 1)
                # drain any remaining fillers at the batch boundary
                emit_fill(fill, 10**6)

            # epilogue: last pair + output projection of the last batch
            ctx_pair(bc - 1, 10)
            for tt in range(TT):
                for _ in o_tile_steps(bc - 1, tt, alt=(tt % 2 == 1)):
                    pass

    return nc


# ---------------------------------------------------------------- host prep
def _prep_shared(Wq, bq, Wk, bk, Wv, bv, Wo, bo):
    """Build the per-core-identical weight operands."""
    scale = np.float32(1.0 / np.sqrt(DH))
    wqf = (Wq.astype(np.float32) * scale).transpose(1, 0, 2).reshape(D, D)
    wkf = Wk.astype(np.float32).transpose(1, 0, 2).reshape(D, D)
    wvf = Wv.astype(np.float32).transpose(1, 0, 2).reshape(D, D)

    def chunk4(wf):  # [d, f] -> [di, m, dc, fi]
        return wf.reshape(DC, 128, FC, 128).transpose(1, 2, 0, 3)

    wqk = np.concatenate([chunk4(wqf), chunk4(wkf)], axis=1)  # [128, 12, 6, 128]
    wv3 = wvf.reshape(DC, 128, D).transpose(1, 0, 2)          # [128, 6, 768]
    wo3 = Wo.astype(np.float32).reshape(FC, 128, D).transpose(1, 0, 2)

    bqf = (bq.astype(np.float32) * scale).reshape(D)
    bkf = bk.astype(np.float32).reshape(D)
    bqk = np.concatenate(
        [bqf.reshape(FC, 128), bkf.reshape(FC, 128)], axis=0
    ).T.copy()                                                # [128, 12]
    bvbc = np.broadcast_to(bv.astype(np.float32).reshape(D), (128, D)).copy()
    bobc = np.broadcast_to(bo.astype(np.float32).reshape(D), (128, D)).copy()

    return {
        "wqk": np.ascontiguousarray(wqk).astype(nbf),
        "wv": np.ascontiguousarray(wv3).astype(nbf),
        "wo": np.ascontiguousarray(wo3).astype(nbf),
        "bqk": np.ascontiguousarray(bqk),
        "bvbc": bvbc,
        "bobc": bobc,
    }


_NC_CACHE = {}


def kernel(x, Wq, bq, Wk, bk, Wv, bv, Wo, bo):
    x = np.asarray(x, dtype=np.float32)
    shared = _prep_shared(
        np.asarray(Wq), np.asarray(bq), np.asarray(Wk), np.asarray(bk),
        np.asarray(Wv), np.asarray(bv), np.asarray(Wo), np.asarray(bo))

    in_maps = []
    for c in range(NCORES):
        xc = x[c * BC : (c + 1) * BC]                    # [BC, S, D]
        xt = xc.transpose(2, 0, 1)                       # [D, BC, S]
        xt = xt.reshape(DC, 128, BC, S).astype(nbf)
        m = dict(shared)
        m["xt"] = np.ascontiguousarray(xt)
        in_maps.append(m)

    if "nc" not in _NC_CACHE:
        _NC_CACHE["nc"] = build_bass()
    nc = _NC_CACHE["nc"]

    res = run_bass_kernel_spmd(nc, in_maps, core_ids=list(range(NCORES)))
    out = np.concatenate([res.results[c]["out"] for c in range(NCORES)], axis=0)
    return out.astype(np.float32)


if __name__ == "__main__":
    rng = np.random.default_rng(0)
    ins = {
        "x": rng.standard_normal((B, S, D), dtype=np.float32),
        "Wq": rng.standard_normal((H, D, DH), dtype=np.float32) * 0.02,
        "bq": np.zeros((H, DH), np.float32),
        "Wk": rng.standard_normal((H, D, DH), dtype=np.float32) * 0.02,
        "bk": np.zeros((H, DH), np.float32),
        "Wv": rng.standard_normal((H, D, DH), dtype=np.float32) * 0.02,
        "bv": np.zeros((H, DH), np.float32),
        "Wo": rng.standard_normal((D, D), dtype=np.float32) * 0.02,
        "bo": np.zeros((D,), np.float32),
    }
    o = kernel(**ins)
    print("out", o.shape, o.dtype, float(np.abs(o).max()))


# revision 24
# speedup vs baseline: 1.0326x; 1.0145x over previous
"""Trainium2 Bass kernel for nn_MultiHeadAttention_31542239822105.

Math (faithful to reference, incl. softmax over the QUERY axis):
  q = einsum('bsd,hde->bhse', x, Wq) + bq ; same k, v
  scores = q @ k^T * 1/sqrt(DH)          [B,H,Sq,Sk]
  probs  = softmax(scores, axis=2)       # over q (query axis!)
  ctx    = einsum('bhqk,bhke->bhqe', probs, v)
  out    = ctx.reshape(B,S,D) @ Wo + bo

Sharding: data-parallel over batch, 8 cores x 8 batch items. No collectives.

Per-core layout strategy (all matmul contraction dims land on partitions):
  - x is pre-transposed on the HOST to xT [D, tokens] so no on-chip transposes.
  - Q^T,K^T come out of the projection f-major ([feat, token]) with W as the
    stationary operand; V comes out token-major with xT as stationary.
  - scoresT[k,q] = K^T.T @ Q^T per head -> softmax over q is a FREE-axis
    reduction; exp+sum fused into the PSUM eviction on ScalarE (accum_out).
  - 1/denominator is folded into V rows (cheap: S*DH vs S*S elements).
  - ctx for a HEAD PAIR runs in (128,64) column-tiled PE mode: the two heads'
    matmuls occupy disjoint 64-column strips of the array concurrently
    (tile_position (0,0)/(0,64)), so the pair costs one head's stream time
    and no zero-padded V operand is needed.
  - output projection uses ctxT chunks as stationary -> token-major result,
    direct DMA out. 1/sqrt(DH) folded into Wq/bq on the host.

Global software pipeline (the main perf trick vs the naive phase-serial
form): ScalarE's exp chain is the serial bottleneck of the attention phase
(~830ns per [128,577] tile while the PE needs only ~500ns to produce it), so
the PE stream for batch b's attention is interleaved at ~250ns granularity
with "filler" matmuls: the output projection of batch b-1 and the Q/K/V
projections of batch b+1. The PE then never idles long enough to drop out of
its fast-clock state, and each batch costs max(PE, Scalar) instead of
PE_proj + Scalar_attn.

PSUM budget (8 banks): scores pool 2 bufs x 2 banks, ctx-pair pool 1 x 2,
projection pool 2 x 1 (projections write a 512-wide and a tail piece into
separate single-bank tiles so two chunks can be in flight).
"""

import sys

if "/opt/trn_rl_repo" not in sys.path:
    sys.path.insert(0, "/opt/trn_rl_repo")

import numpy as np
import ml_dtypes

import concourse.bass as bass
import concourse.mybir as mybir
import concourse.tile as tile_mod
from concourse.vector_clock import ScopedClock
from concourse.bass_utils import run_bass_kernel_spmd

# ---------------------------------------------------------------- constants
B, S, D, H = 64, 577, 768, 12
DH = D // H          # 64
NCORES = 8
BC = B // NCORES     # 8 batch items per core
DC = D // 128        # 6 d-chunks
FC = D // 128        # 6 f-chunks per projection matrix
M_QK = 2 * FC        # 12 combined Q+K f-chunks
TT = (S + 127) // 128  # 5 token tiles (128,128,128,128,65)
S0 = 512             # PSUM-bank-sized free-dim split: 577 = 512 + 65
S1 = S - S0

BF16 = mybir.dt.bfloat16
F32 = mybir.dt.float32
nbf = ml_dtypes.bfloat16

_TILE_PATCHED = False
_CUR_NC = [None]


def _patch_tile_drain():
    """The walrus build here rejects >1 sync-wait per instruction
    ("Too many sync wait commands"). Two patches:
    1. post-legalize pass that moves extra waits onto single-wait nops
       inserted just before the offending instruction (same engine);
    2. the final SP Drain (emitted after legalize) gets the same split.
    """
    global _TILE_PATCHED
    if _TILE_PATCHED:
        return
    _TILE_PATCHED = True

    _orig_postorder = tile_mod.postorder_instruction_blocks

    def _split_multi_waits(ordered, nc):
        for bbname, insts in ordered.items():
            out = []
            n_split = 0
            for inst in insts:
                si = inst.sync_info
                if si is not None and len(si.on_wait) > 1:
                    waits = list(si.on_wait)
                    for w in waits[:-1]:
                        nop = mybir.InstNoOp(
                            name=nc.get_next_instruction_name(),
                            ins=[],
                            outs=[],
                            bass_is_fusable=False,
                        )
                        nop.engine = inst.engine
                        nop.sync_info = mybir.SyncInfo(on_wait=[w], on_update=[])
                        nc.register_instruction(nop, overwrite=True)
                        out.append(nop)
                        n_split += 1
                    inst.sync_info = mybir.SyncInfo(
                        on_wait=[waits[-1]], on_update=list(si.on_update)
                    )
                out.append(inst)
            ordered[bbname] = out
        return ordered

    def postorder_and_split(ordered, start_bb, postordered):
        # Runs post-sem-assignment, right before lowering: the only spot
        # where the final per-instruction waits are visible and editable.
        nc = _CUR_NC[0]
        _split_multi_waits(ordered, nc)
        return _orig_postorder(ordered, start_bb, postordered)

    tile_mod.postorder_instruction_blocks = postorder_and_split

    def _drain_and_barrier_split(self, tick_clock, wait_clock):
        nc = self.nc
        drain_inst = nc.sync.drain()
        wait_clock.add_sem_waits(
            drain_inst.ins, ScopedClock({None: tick_clock.global_clock})
        )
        si = drain_inst.ins.sync_info
        waits = list(si.on_wait)
        if len(waits) > 1:
            drain_inst.ins.sync_info = mybir.SyncInfo(
                on_wait=[waits[0]], on_update=list(si.on_update)
            )
            for w in waits[1:]:
                nop = nc.sync.nop(nofuse=True)
                nop.ins.sync_info = mybir.SyncInfo(on_wait=[w], on_update=[])
        nc.all_engine_barrier()
        assert self.sems is not None
        popped = nc._tile_sem_poison_stack.pop()
        assert popped is self._sem_poison
        nc.clear_and_free_semaphores(list(self.sems.allocated().values()))
        nc.all_engine_barrier()

    tile_mod.TileContext._drain_and_barrier = _drain_and_barrier_split


# ---------------------------------------------------------------- builder
def build_bass(bc=BC):
    """Emit the per-core kernel for `bc` batch items. Returns nc."""
    _patch_tile_drain()
    nc = bass.Bass()
    _CUR_NC[0] = nc

    xt_d = nc.declare_dram_parameter("xt", [DC, 128, bc, S], BF16, isOutput=False)
    wqk_d = nc.declare_dram_parameter("wqk", [128, M_QK, DC, 128], BF16, isOutput=False)
    wv_d = nc.declare_dram_parameter("wv", [128, DC, D], BF16, isOutput=False)
    wo_d = nc.declare_dram_parameter("wo", [128, FC, D], BF16, isOutput=False)
    bqk_d = nc.declare_dram_parameter("bqk", [128, M_QK], F32, isOutput=False)
    bvbc_d = nc.declare_dram_parameter("bvbc", [128, D], F32, isOutput=False)
    bobc_d = nc.declare_dram_parameter("bobc", [128, D], F32, isOutput=False)
    out_d = nc.declare_dram_parameter("out", [bc, S, D], F32, isOutput=True)

    AF = mybir.ActivationFunctionType

    with tile_mod.TileContext(nc) as tc:
        with (
            tc.tile_pool(name="singles", bufs=1) as singles,
            tc.tile_pool(name="xt", bufs=2) as xpool,
            tc.tile_pool(name="qk", bufs=2) as qkpool,
            tc.tile_pool(name="ktz", bufs=2) as kzpool,
            tc.tile_pool(name="v", bufs=2) as vpool,
            tc.tile_pool(name="probs", bufs=4) as ppool,
            tc.tile_pool(name="den", bufs=4) as dpool,
            tc.tile_pool(name="rd", bufs=4) as rdpool,
            tc.tile_pool(name="vs", bufs=4) as vspool,
            tc.tile_pool(name="ctx", bufs=2) as cpool,
            tc.tile_pool(name="ot", bufs=3) as opool,
            tc.tile_pool(name="psc", bufs=2, space="PSUM") as psc,
            tc.tile_pool(name="pcx", bufs=1, space="PSUM") as pcx,
            tc.tile_pool(name="ppj", bufs=2, space="PSUM") as ppj,
        ):
            # -------- resident weights / biases (tiles only; DMAs are
            # issued after batch 0's x so the first matmul starts early)
            wqk = singles.tile([128, M_QK, DC, 128], BF16)
            wv = singles.tile([128, DC, D], BF16)
            wo = singles.tile([128, FC, D], BF16)
            bqk = singles.tile([128, M_QK], F32)
            bvbc = singles.tile([128, D], F32)
            bobc = singles.tile([128, D], F32)

            bat = {}

            def start_batch(b):
                xt = xpool.tile([128, DC, S], BF16, tag="xt", name=f"xt{b}")
                for dc in range(DC):
                    nc.sync.dma_start(out=xt[:, dc, :], in_=xt_d[dc, :, b, :])
                bat[b] = dict(
                    xt=xt,
                    qk=qkpool.tile([128, FC, S], BF16, tag="qk", name=f"qk{b}"),
                    ktz=kzpool.tile(
                        [128, FC, 2, S], BF16, tag="ktz", name=f"ktz{b}"
                    ),
                    v=vpool.tile([128, TT, D], BF16, tag="v", name=f"v{b}"),
                    ctxT=cpool.tile([128, FC, S], BF16, tag="ctx", name=f"ctxT{b}"),
                    probs={}, den={}, vs={},
                )

            # ---- projection filler generators (each yield = ~1 PE dc-step)
            def qk_chunk_steps(b, m, alt=False):
                # alt=True (prologue/epilogue only): use the idle scores
                # pool so twice as many chunks can be in flight.
                Bt = bat[b]
                if alt:
                    ps = psc.tile([128, S], F32, tag="sc", name=f"qP{b}_{m}")
                    psA, psB = ps[:, 0:S0], ps[:, S0:S]
                else:
                    psA = ppj.tile([128, S0], F32, tag="pj", name=f"qA{b}_{m}")
                    psB = ppj.tile([128, S1], F32, tag="pj", name=f"qB{b}_{m}")
                for dc in range(DC):
                    st, sp = dc == 0, dc == DC - 1
                    nc.tensor.matmul(
                        psA, lhsT=wqk[:, m, dc, :], rhs=Bt["xt"][:, dc, 0:S0],
                        start=st, stop=sp)
                    nc.tensor.matmul(
                        psB, lhsT=wqk[:, m, dc, :], rhs=Bt["xt"][:, dc, S0:S],
                        start=st, stop=sp)
                    if sp:
                        # In the prologue (b==0) ScalarE is idle and VectorE
                        # is the eviction bottleneck -> split evictions.
                        # In steady state ScalarE is the exp chain -> keep
                        # everything on VectorE.
                        if m < FC:
                            if b == 0:
                                nc.scalar.activation(
                                    Bt["qk"][:, m, 0:S0], psA, AF.Identity,
                                    bias=bqk[:, m : m + 1], scale=1.0)
                                nc.scalar.activation(
                                    Bt["qk"][:, m, S0:S], psB, AF.Identity,
                                    bias=bqk[:, m : m + 1], scale=1.0)
                            else:
                                nc.vector.tensor_scalar_add(
                                    Bt["qk"][:, m, 0:S0], psA, bqk[:, m : m + 1])
                                nc.vector.tensor_scalar_add(
                                    Bt["qk"][:, m, S0:S], psB, bqk[:, m : m + 1])
                        else:
                            mk = m - FC
                            if b < 2:
                                # zero halves persist across pool reuse:
                                # only the first user of each buffer pays.
                                nc.vector.memset(Bt["ktz"][64:128, mk, 0, :], 0.0)
                                nc.vector.memset(Bt["ktz"][0:64, mk, 1, :], 0.0)
                            if b == 0:
                                nc.scalar.activation(
                                    Bt["ktz"][0:64, mk, 0, 0:S0], psA[0:64],
                                    AF.Identity, bias=bqk[0:64, m : m + 1],
                                    scale=1.0)
                                nc.scalar.activation(
                                    Bt["ktz"][0:64, mk, 0, S0:S], psB[0:64],
                                    AF.Identity, bias=bqk[0:64, m : m + 1],
                                    scale=1.0)
                            else:
                                nc.vector.tensor_scalar_add(
                                    Bt["ktz"][0:64, mk, 0, 0:S0], psA[0:64],
                                    bqk[0:64, m : m + 1])
                                nc.vector.tensor_scalar_add(
                                    Bt["ktz"][0:64, mk, 0, S0:S], psB[0:64],
                                    bqk[0:64, m : m + 1])
                            nc.vector.tensor_scalar_add(
                                Bt["ktz"][64:128, mk, 1, 0:S0], psA[64:128],
                                bqk[64:128, m : m + 1])
                            nc.vector.tensor_scalar_add(
                                Bt["ktz"][64:128, mk, 1, S0:S], psB[64:128],
                                bqk[64:128, m : m + 1])
                    yield

            def v_tile_steps(b, tt, alt=False):
                Bt = bat[b]
                tsz = min(128, S - tt * 128)
                t0 = tt * 128
                if alt:
                    ps = pcx.tile([128, D], F32, tag="cx", name=f"vP{b}_{tt}")
                    psA, psB = ps[:, 0:512], ps[:, 512:D]
                else:
                    psA = ppj.tile([128, 512], F32, tag="pj", name=f"vA{b}_{tt}")
                    psB = ppj.tile([128, 256], F32, tag="pj", name=f"vB{b}_{tt}")
                for dc in range(DC):
                    st, sp = dc == 0, dc == DC - 1
                    nc.tensor.matmul(
                        psA[:tsz], lhsT=Bt["xt"][:, dc, t0 : t0 + tsz],
                        rhs=wv[:, dc, 0:512], start=st, stop=sp)
                    nc.tensor.matmul(
                        psB[:tsz], lhsT=Bt["xt"][:, dc, t0 : t0 + tsz],
                        rhs=wv[:, dc, 512:D], start=st, stop=sp)
                    if sp:
                        nc.vector.tensor_add(
                            Bt["v"][:tsz, tt, 0:512], psA[:tsz], bvbc[:tsz, 0:512])
                        nc.vector.tensor_add(
                            Bt["v"][:tsz, tt, 512:D], psB[:tsz], bvbc[:tsz, 512:D])
                    yield

            def o_tile_steps(b, tt, alt=False):
                Bt = bat[b]
                tsz = min(128, S - tt * 128)
                t0 = tt * 128
                if alt:
                    ps = pcx.tile([128, D], F32, tag="cx", name=f"oP{b}_{tt}")
                    psA, psB = ps[:, 0:512], ps[:, 512:D]
                else:
                    psA = ppj.tile([128, 512], F32, tag="pj", name=f"oA{b}_{tt}")
                    psB = ppj.tile([128, 256], F32, tag="pj", name=f"oB{b}_{tt}")
                for fc in range(FC):
                    st, sp = fc == 0, fc == FC - 1
                    nc.tensor.matmul(
                        psA[:tsz], lhsT=Bt["ctxT"][:, fc, t0 : t0 + tsz],
                        rhs=wo[:, fc, 0:512], start=st, stop=sp)
                    nc.tensor.matmul(
                        psB[:tsz], lhsT=Bt["ctxT"][:, fc, t0 : t0 + tsz],
                        rhs=wo[:, fc, 512:D], start=st, stop=sp)
                    if sp:
                        ot = opool.tile([128, D], F32, tag="ot", name=f"ot{b}_{tt}")
                        nc.vector.tensor_add(
                            ot[:tsz, 0:512], psA[:tsz], bobc[:tsz, 0:512])
                        nc.vector.tensor_add(
                            ot[:tsz, 512:D], psB[:tsz], bobc[:tsz, 512:D])
                        nc.sync.dma_start(
                            out=out_d[b, t0 : t0 + tsz, :], in_=ot[:tsz])
                    yield

            def gen_fill(b):
                """Filler steps emitted during attention(b): output
                projection of b-1 and Q/K/V projections of b+1, chunk-
                sequential (the pj psum pool holds one chunk's two pieces)."""
                gens = []
                qgens = []
                if b + 1 < bc:
                    start_batch(b + 1)
                    # pair Q and K chunks so head h's operands finish early
                    order = []
                    for i in range(FC):
                        order += [i, FC + i]
                    qgens = [qk_chunk_steps(b + 1, m) for m in order]
                ogens = [o_tile_steps(b - 1, tt) for tt in range(TT)] if b >= 1 else []
                # interleave at chunk granularity: q, o, q, o, ... then v
                qi, oi = iter(qgens), iter(ogens)
                while True:
                    qn = next(qi, None)
                    on = next(oi, None)
                    if qn is None and on is None:
                        break
                    if qn is not None:
                        gens.append(qn)
                    if on is not None:
                        gens.append(on)
                if b + 1 < bc:
                    gens += [v_tile_steps(b + 1, tt) for tt in range(TT)]
                for g in gens:
                    yield from g

            # ---- attention pieces
            def sc_step(b, h, kc):
                Bt = bat[b]
                m, hh = h // 2, h % 2
                ksz = min(128, S - kc * 128)
                k0 = kc * 128
                ps = psc.tile([128, S], F32, tag="sc", name=f"sc{b}_{h}_{kc}")
                nc.tensor.matmul(
                    ps[:ksz, 0:S0], lhsT=Bt["ktz"][:, m, hh, k0 : k0 + ksz],
                    rhs=Bt["qk"][:, m, 0:S0], start=True, stop=True)
                nc.tensor.matmul(
                    ps[:ksz, S0:S], lhsT=Bt["ktz"][:, m, hh, k0 : k0 + ksz],
                    rhs=Bt["qk"][:, m, S0:S], start=True, stop=True)
                nc.scalar.activation(
                    Bt["probs"][h][:ksz, kc, :], ps[:ksz, 0:S], AF.Exp,
                    accum_out=Bt["den"][h][:ksz, kc : kc + 1])

            def vs_prep(b, h):
                """1/den folded into this head's V columns, zero-padded to a
                full 128-col stationary operand (sub-128 tiling modes drop
                the PE out of its fast clock, so ctx uses full-array
                matmuls with the other head's half zeroed). The pad halves
                persist across pool reuse: 12 heads % 4 bufs = 0, so each
                buffer always carries the same head parity."""
                Bt = bat[b]
                rd = rdpool.tile([128, TT], F32, tag="rd", name=f"rd{b}_{h}")
                nc.vector.reciprocal(rd, Bt["den"][h])
                po = (h % 2) * 64
                vs = vspool.tile([128, TT, 128], BF16, tag="vs", name=f"vs{b}_{h}")
                if b == 0 and h < 4:
                    nc.vector.memset(vs[:, :, 64 - po : 128 - po], 0.0)
                for kc in range(TT):
                    ksz = min(128, S - kc * 128)
                    nc.vector.tensor_scalar_mul(
                        vs[:ksz, kc, po : po + DH],
                        Bt["v"][:ksz, kc, h * DH : (h + 1) * DH],
                        rd[:ksz, kc : kc + 1])
                Bt["vs"][h] = vs

            def ctx_pair(b, h0, fill=None):
                """ctx for heads (h0, h0+1): both accumulate into one
                [128, S] PSUM tile (head h0 owns partitions 0:64, h0+1 owns
                64:128 via the zero-padded halves of vs). At a batch
                boundary head h0+1's probs are still draining from ScalarE,
                so the caller passes `fill` to pad between the halves."""
                Bt = bat[b]
                ps = pcx.tile([128, S], F32, tag="cx", name=f"cx{b}_{h0}")
                for hh in (h0, h0 + 1):
                    for kc in range(TT):
                        ksz = min(128, S - kc * 128)
                        st = hh == h0 and kc == 0
                        sp = hh == h0 + 1 and kc == TT - 1
                        nc.tensor.matmul(
                            ps[:, 0:S0], lhsT=Bt["vs"][hh][:ksz, kc, :],
                            rhs=Bt["probs"][hh][:ksz, kc, 0:S0],
                            start=st, stop=sp)
                        nc.tensor.matmul(
                            ps[:, S0:S], lhsT=Bt["vs"][hh][:ksz, kc, :],
                            rhs=Bt["probs"][hh][:ksz, kc, S0:S],
                            start=st, stop=sp)
                    if hh == h0 and fill is not None:
                        emit_fill(fill, 3)
                nc.vector.tensor_copy(Bt["ctxT"][:, h0 // 2, :], ps[:, 0:S])
                # pair's probs/vs no longer needed; let pools rotate
                for hh in (h0, h0 + 1):
                    Bt["probs"].pop(hh, None)
                    Bt["vs"].pop(hh, None)

            # ---- the schedule
            def emit_fill(fill, n):
                for _ in range(n):
                    try:
                        next(fill)
                    except StopIteration:
                        return

            def slot(b, h, fill):
                Bt = bat[b]
                if h >= 1:
                    vs_prep(b, h - 1)
                probs = ppool.tile([128, TT, S], BF16, tag="probs",
                                   name=f"pr{b}_{h}")
                den = dpool.tile([128, TT], F32, tag="den", name=f"dn{b}_{h}")
                nc.vector.memset(den, 1.0)
                Bt["probs"][h] = probs
                Bt["den"][h] = den
                # paced emission: scores tiles gated by the exp chain two
                # tiles back (scores psum pool bufs=2); fillers keep the PE
                # fed while ScalarE drains.
                if h == 0:
                    # batch boundary: ScalarE is draining the previous
                    # batch's trailing exps. Issue next-batch DMAs + filler
                    # and the carry-over ctx pair BEFORE touching the scores
                    # psum pool so the PE never waits on that drain.
                    emit_fill(fill, 1)
                    if b >= 1:
                        ctx_pair(b - 1, 10, fill)
                    emit_fill(fill, 1)
                    sc_step(b, h, 0)
                    emit_fill(fill, 1); sc_step(b, h, 1)
                    emit_fill(fill, 1); sc_step(b, h, 2)
                    emit_fill(fill, 1); sc_step(b, h, 3)
                    emit_fill(fill, 1); sc_step(b, h, 4)
                    emit_fill(fill, 1)
                elif h == 1:
                    # the slot after a batch boundary inherits ScalarE
                    # backlog: front-load the fillers
                    emit_fill(fill, 2); sc_step(b, h, 0)
                    emit_fill(fill, 3); sc_step(b, h, 1)
                    emit_fill(fill, 3); sc_step(b, h, 2)
                    emit_fill(fill, 3); sc_step(b, h, 3)
                    emit_fill(fill, 3); sc_step(b, h, 4)
                    emit_fill(fill, 3)
                elif h % 2 == 1:
                    sc_step(b, h, 0)
                    emit_fill(fill, 2); sc_step(b, h, 1)
                    emit_fill(fill, 2); sc_step(b, h, 2)
                    emit_fill(fill, 3); sc_step(b, h, 3)
                    emit_fill(fill, 3); sc_step(b, h, 4)
                    emit_fill(fill, 5)
                else:
                    sc_step(b, h, 0)
                    emit_fill(fill, 1); sc_step(b, h, 1)
                    emit_fill(fill, 1); sc_step(b, h, 2)
                    ctx_pair(b, h - 2)
                    sc_step(b, h, 3)
                    emit_fill(fill, 2); sc_step(b, h, 4)
                    emit_fill(fill, 2)

            # prologue: batch 0 projections, nothing to overlap with.
            # DMA order: first QK weights + x(0) (the first matmul's
            # operands), then everything needed later.
            nc.sync.dma_start(out=bqk, in_=bqk_d[:])
            nc.sync.dma_start(out=wqk, in_=wqk_d[:])
            start_batch(0)
            nc.sync.dma_start(out=bvbc, in_=bvbc_d[:])
            nc.sync.dma_start(out=wv, in_=wv_d[:])
            nc.sync.dma_start(out=wo, in_=wo_d[:])
            nc.sync.dma_start(out=bobc, in_=bobc_d[:])
            order0 = []
            for i in range(FC):
                order0 += [i, FC + i]
            for idx, m in enumerate(order0):
                for _ in qk_chunk_steps(0, m, alt=(idx % 2 == 0)):
                    pass
            for tt in range(TT):
                for _ in v_tile_steps(0, tt, alt=(tt % 2 == 0)):
                    pass

            def throttled(gen, k):
                """Yield k times per real filler step so a short filler
                stream spreads over the whole batch (last batch has only
                the previous batch's output projection to offer)."""
                while True:
                    try:
                        next(gen)
                    except StopIteration:
                        return
                    for _ in range(k - 1):
                        yield
                    yield

            # main loop
            for b in range(bc):
                fill = gen_fill(b)
                if b == bc - 1:
                    fill = throttled(fill, 4)
                for h in range(H):
                    slot(b, h, fill)
                vs_prep(b, H - 1)
                # drain any remaining fillers at the batch boundary
                emit_fill(fill, 10**6)

            # epilogue: last pair + output projection of the last batch
            ctx_pair(bc - 1, 10)
            for tt in range(TT):
                for _ in o_tile_steps(bc - 1, tt, alt=(tt % 2 == 1)):
                    pass

    return nc


# ---------------------------------------------------------------- host prep
def _prep_shared(Wq, bq, Wk, bk, Wv, bv, Wo, bo):
    """Build the per-core-identical weight operands."""
    scale = np.float32(1.0 / np.sqrt(DH))
    wqf = (Wq.astype(np.float32) * scale).transpose(1, 0, 2).reshape(D, D)
    wkf = Wk.astype(np.float32).transpose(1, 0, 2).reshape(D, D)
    wvf = Wv.astype(np.float32).transpose(1, 0, 2).reshape(D, D)

    def chunk4(wf):  # [d, f] -> [di, m, dc, fi]
        return wf.reshape(DC, 128, FC, 128).transpose(1, 2, 0, 3)

    wqk = np.concatenate([chunk4(wqf), chunk4(wkf)], axis=1)  # [128, 12, 6, 128]
    wv3 = wvf.reshape(DC, 128, D).transpose(1, 0, 2)          # [128, 6, 768]
    wo3 = Wo.astype(np.float32).reshape(FC, 128, D).transpose(1, 0, 2)

    bqf = (bq.astype(np.float32) * scale).reshape(D)
    bkf = bk.astype(np.float32).reshape(D)
    bqk = np.concatenate(
        [bqf.reshape(FC, 128), bkf.reshape(FC, 128)], axis=0
    ).T.copy()                                                # [128, 12]
    bvbc = np.broadcast_to(bv.astype(np.float32).reshape(D), (128, D)).copy()
    bobc = np.broadcast_to(bo.astype(np.float32).reshape(D), (128, D)).copy()

    return {
        "wqk": np.ascontiguousarray(wqk).astype(nbf),
        "wv": np.ascontiguousarray(wv3).astype(nbf),
        "wo": np.ascontiguousarray(wo3).astype(nbf),
        "bqk": np.ascontiguousarray(bqk),
        "bvbc": bvbc,
        "bobc": bobc,
    }


_NC_CACHE = {}


def kernel(x, Wq, bq, Wk, bk, Wv, bv, Wo, bo):
    x = np.asarray(x, dtype=np.float32)
    shared = _prep_shared(
        np.asarray(Wq), np.asarray(bq), np.asarray(Wk), np.asarray(bk),
        np.asarray(Wv), np.asarray(bv), np.asarray(Wo), np.asarray(bo))

    in_maps = []
    for c in range(NCORES):
        xc = x[c * BC : (c + 1) * BC]                    # [BC, S, D]
        xt = xc.transpose(2, 0, 1)                       # [D, BC, S]
        xt = xt.reshape(DC, 128, BC, S).astype(nbf)
        m = dict(shared)
        m["xt"] = np.ascontiguousarray(xt)
        in_maps.append(m)

    if "nc" not in _NC_CACHE:
        _NC_CACHE["nc"] = build_bass()
    nc = _NC_CACHE["nc"]

    res = run_bass_kernel_spmd(nc, in_maps, core_ids=list(range(NCORES)))
    out = np.concatenate([res.results[c]["out"] for c in range(NCORES)], axis=0)
    return out.astype(np.float32)


if __name__ == "__main__":
    rng = np.random.default_rng(0)
    ins = {
        "x": rng.standard_normal((B, S, D), dtype=np.float32),
        "Wq": rng.standard_normal((H, D, DH), dtype=np.float32) * 0.02,
        "bq": np.zeros((H, DH), np.float32),
        "Wk": rng.standard_normal((H, D, DH), dtype=np.float32) * 0.02,
        "bk": np.zeros((H, DH), np.float32),
        "Wv": rng.standard_normal((H, D, DH), dtype=np.float32) * 0.02,
        "bv": np.zeros((H, DH), np.float32),
        "Wo": rng.standard_normal((D, D), dtype=np.float32) * 0.02,
        "bo": np.zeros((D,), np.float32),
    }
    o = kernel(**ins)
    print("out", o.shape, o.dtype, float(np.abs(o).max()))


# revision 28
# speedup vs baseline: 1.0453x; 1.0123x over previous
"""Trainium2 Bass kernel for nn_MultiHeadAttention_31542239822105.

Math (faithful to reference, incl. softmax over the QUERY axis):
  q = einsum('bsd,hde->bhse', x, Wq) + bq ; same k, v
  scores = q @ k^T * 1/sqrt(DH)          [B,H,Sq,Sk]
  probs  = softmax(scores, axis=2)       # over q (query axis!)
  ctx    = einsum('bhqk,bhke->bhqe', probs, v)
  out    = ctx.reshape(B,S,D) @ Wo + bo

Sharding: data-parallel over batch, 8 cores x 8 batch items. No collectives.

Per-core layout strategy (all matmul contraction dims land on partitions):
  - x is pre-transposed on the HOST to xT [D, tokens] so no on-chip transposes.
  - Q^T,K^T come out of the projection f-major ([feat, token]) with W as the
    stationary operand; V comes out token-major with xT as stationary.
  - scoresT[k,q] = K^T.T @ Q^T per head -> softmax over q is a FREE-axis
    reduction; exp+sum fused into the PSUM eviction on ScalarE (accum_out).
  - 1/denominator is folded into V rows (cheap: S*DH vs S*S elements).
  - ctx for a HEAD PAIR runs in (128,64) column-tiled PE mode: the two heads'
    matmuls occupy disjoint 64-column strips of the array concurrently
    (tile_position (0,0)/(0,64)), so the pair costs one head's stream time
    and no zero-padded V operand is needed.
  - output projection uses ctxT chunks as stationary -> token-major result,
    direct DMA out. 1/sqrt(DH) folded into Wq/bq on the host.

Global software pipeline (the main perf trick vs the naive phase-serial
form): ScalarE's exp chain is the serial bottleneck of the attention phase
(~830ns per [128,577] tile while the PE needs only ~500ns to produce it), so
the PE stream for batch b's attention is interleaved at ~250ns granularity
with "filler" matmuls: the output projection of batch b-1 and the Q/K/V
projections of batch b+1. The PE then never idles long enough to drop out of
its fast-clock state, and each batch costs max(PE, Scalar) instead of
PE_proj + Scalar_attn.

PSUM budget (8 banks): scores pool 2 bufs x 2 banks, ctx-pair pool 1 x 2,
projection pool 2 x 1 (projections write a 512-wide and a tail piece into
separate single-bank tiles so two chunks can be in flight).
"""

import sys

if "/opt/trn_rl_repo" not in sys.path:
    sys.path.insert(0, "/opt/trn_rl_repo")

import numpy as np
import ml_dtypes

import concourse.bass as bass
import concourse.mybir as mybir
import concourse.tile as tile_mod
from concourse.vector_clock import ScopedClock
from concourse.bass_utils import run_bass_kernel_spmd

# ---------------------------------------------------------------- constants
B, S, D, H = 64, 577, 768, 12
DH = D // H          # 64
NCORES = 8
BC = B // NCORES     # 8 batch items per core
DC = D // 128        # 6 d-chunks
FC = D // 128        # 6 f-chunks per projection matrix
M_QK = 2 * FC        # 12 combined Q+K f-chunks
TT = (S + 127) // 128  # 5 token tiles (128,128,128,128,65)
S0 = 512             # PSUM-bank-sized free-dim split: 577 = 512 + 65
S1 = S - S0

BF16 = mybir.dt.bfloat16
F32 = mybir.dt.float32
nbf = ml_dtypes.bfloat16

_TILE_PATCHED = False
_CUR_NC = [None]


def _patch_tile_drain():
    """The walrus build here rejects >1 sync-wait per instruction
    ("Too many sync wait commands"). Two patches:
    1. post-legalize pass that moves extra waits onto single-wait nops
       inserted just before the offending instruction (same engine);
    2. the final SP Drain (emitted after legalize) gets the same split.
    """
    global _TILE_PATCHED
    if _TILE_PATCHED:
        return
    _TILE_PATCHED = True

    _orig_postorder = tile_mod.postorder_instruction_blocks

    def _split_multi_waits(ordered, nc):
        for bbname, insts in ordered.items():
            out = []
            n_split = 0
            for inst in insts:
                si = inst.sync_info
                if si is not None and len(si.on_wait) > 1:
                    waits = list(si.on_wait)
                    for w in waits[:-1]:
                        nop = mybir.InstNoOp(
                            name=nc.get_next_instruction_name(),
                            ins=[],
                            outs=[],
                            bass_is_fusable=False,
                        )
                        nop.engine = inst.engine
                        nop.sync_info = mybir.SyncInfo(on_wait=[w], on_update=[])
                        nc.register_instruction(nop, overwrite=True)
                        out.append(nop)
                        n_split += 1
                    inst.sync_info = mybir.SyncInfo(
                        on_wait=[waits[-1]], on_update=list(si.on_update)
                    )
                out.append(inst)
            ordered[bbname] = out
        return ordered

    def postorder_and_split(ordered, start_bb, postordered):
        # Runs post-sem-assignment, right before lowering: the only spot
        # where the final per-instruction waits are visible and editable.
        nc = _CUR_NC[0]
        _split_multi_waits(ordered, nc)
        return _orig_postorder(ordered, start_bb, postordered)

    tile_mod.postorder_instruction_blocks = postorder_and_split

    def _drain_and_barrier_split(self, tick_clock, wait_clock):
        nc = self.nc
        drain_inst = nc.sync.drain()
        wait_clock.add_sem_waits(
            drain_inst.ins, ScopedClock({None: tick_clock.global_clock})
        )
        si = drain_inst.ins.sync_info
        waits = list(si.on_wait)
        if len(waits) > 1:
            drain_inst.ins.sync_info = mybir.SyncInfo(
                on_wait=[waits[0]], on_update=list(si.on_update)
            )
            for w in waits[1:]:
                nop = nc.sync.nop(nofuse=True)
                nop.ins.sync_info = mybir.SyncInfo(on_wait=[w], on_update=[])
        nc.all_engine_barrier()
        assert self.sems is not None
        popped = nc._tile_sem_poison_stack.pop()
        assert popped is self._sem_poison
        nc.clear_and_free_semaphores(list(self.sems.allocated().values()))
        nc.all_engine_barrier()

    tile_mod.TileContext._drain_and_barrier = _drain_and_barrier_split


# ---------------------------------------------------------------- builder
def build_bass(bc=BC):
    """Emit the per-core kernel for `bc` batch items. Returns nc."""
    _patch_tile_drain()
    nc = bass.Bass()
    _CUR_NC[0] = nc

    xt_d = nc.declare_dram_parameter("xt", [DC, 128, bc, S], BF16, isOutput=False)
    wqk_d = nc.declare_dram_parameter("wqk", [128, M_QK, DC, 128], BF16, isOutput=False)
    wv_d = nc.declare_dram_parameter("wv", [128, DC, D], BF16, isOutput=False)
    wo_d = nc.declare_dram_parameter("wo", [128, FC, D], BF16, isOutput=False)
    bqk_d = nc.declare_dram_parameter("bqk", [128, M_QK], F32, isOutput=False)
    bvbc_d = nc.declare_dram_parameter("bvbc", [128, D], F32, isOutput=False)
    bobc_d = nc.declare_dram_parameter("bobc", [128, D], F32, isOutput=False)
    out_d = nc.declare_dram_parameter("out", [bc, S, D], F32, isOutput=True)

    AF = mybir.ActivationFunctionType

    with tile_mod.TileContext(nc) as tc:
        with (
            tc.tile_pool(name="singles", bufs=1) as singles,
            tc.tile_pool(name="xt", bufs=2) as xpool,
            tc.tile_pool(name="qk", bufs=2) as qkpool,
            tc.tile_pool(name="ktz", bufs=2) as kzpool,
            tc.tile_pool(name="v", bufs=2) as vpool,
            tc.tile_pool(name="probs", bufs=4) as ppool,
            tc.tile_pool(name="den", bufs=4) as dpool,
            tc.tile_pool(name="rd", bufs=4) as rdpool,
            tc.tile_pool(name="vs", bufs=4) as vspool,
            tc.tile_pool(name="ctx", bufs=2) as cpool,
            tc.tile_pool(name="ot", bufs=3) as opool,
            tc.tile_pool(name="psc", bufs=2, space="PSUM") as psc,
            tc.tile_pool(name="pcx", bufs=1, space="PSUM") as pcx,
            tc.tile_pool(name="ppj", bufs=2, space="PSUM") as ppj,
        ):
            # -------- resident weights / biases (tiles only; DMAs are
            # issued after batch 0's x so the first matmul starts early)
            wqk = singles.tile([128, M_QK, DC, 128], BF16)
            wv = singles.tile([128, DC, D], BF16)
            wo = singles.tile([128, FC, D], BF16)
            bqk = singles.tile([128, M_QK], F32)
            bvbc = singles.tile([128, D], F32)
            bobc = singles.tile([128, D], F32)

            bat = {}

            def start_batch(b):
                xt = xpool.tile([128, DC, S], BF16, tag="xt", name=f"xt{b}")
                for dc in range(DC):
                    nc.sync.dma_start(out=xt[:, dc, :], in_=xt_d[dc, :, b, :])
                bat[b] = dict(
                    xt=xt,
                    qk=qkpool.tile([128, FC, S], BF16, tag="qk", name=f"qk{b}"),
                    ktz=kzpool.tile(
                        [128, FC, 2, S], BF16, tag="ktz", name=f"ktz{b}"
                    ),
                    v=vpool.tile([128, TT, D], BF16, tag="v", name=f"v{b}"),
                    ctxT=cpool.tile([128, FC, S], BF16, tag="ctx", name=f"ctxT{b}"),
                    probs={}, den={}, vs={},
                )

            # ---- projection filler generators (each yield = ~1 PE dc-step)
            def qk_chunk_steps(b, m, alt=False):
                # alt=True (prologue/epilogue only): use the idle scores
                # pool so twice as many chunks can be in flight.
                Bt = bat[b]
                if alt:
                    ps = psc.tile([128, S], F32, tag="sc", name=f"qP{b}_{m}")
                    psA, psB = ps[:, 0:S0], ps[:, S0:S]
                else:
                    psA = ppj.tile([128, S0], F32, tag="pj", name=f"qA{b}_{m}")
                    psB = ppj.tile([128, S1], F32, tag="pj", name=f"qB{b}_{m}")
                for dc in range(DC):
                    st, sp = dc == 0, dc == DC - 1
                    nc.tensor.matmul(
                        psA, lhsT=wqk[:, m, dc, :], rhs=Bt["xt"][:, dc, 0:S0],
                        start=st, stop=sp)
                    nc.tensor.matmul(
                        psB, lhsT=wqk[:, m, dc, :], rhs=Bt["xt"][:, dc, S0:S],
                        start=st, stop=sp)
                    if sp:
                        # In the prologue (b==0) ScalarE is idle and VectorE
                        # is the eviction bottleneck -> split evictions.
                        # In steady state ScalarE is the exp chain -> keep
                        # everything on VectorE.
                        if m < FC:
                            if b == 0:
                                nc.scalar.activation(
                                    Bt["qk"][:, m, 0:S0], psA, AF.Identity,
                                    bias=bqk[:, m : m + 1], scale=1.0)
                                nc.scalar.activation(
                                    Bt["qk"][:, m, S0:S], psB, AF.Identity,
                                    bias=bqk[:, m : m + 1], scale=1.0)
                            else:
                                nc.vector.tensor_scalar_add(
                                    Bt["qk"][:, m, 0:S0], psA, bqk[:, m : m + 1])
                                nc.vector.tensor_scalar_add(
                                    Bt["qk"][:, m, S0:S], psB, bqk[:, m : m + 1])
                        else:
                            mk = m - FC
                            if b < 2:
                                # zero halves persist across pool reuse:
                                # only the first user of each buffer pays.
                                nc.vector.memset(Bt["ktz"][64:128, mk, 0, :], 0.0)
                                nc.vector.memset(Bt["ktz"][0:64, mk, 1, :], 0.0)
                            if b == 0:
                                nc.scalar.activation(
                                    Bt["ktz"][0:64, mk, 0, 0:S0], psA[0:64],
                                    AF.Identity, bias=bqk[0:64, m : m + 1],
                                    scale=1.0)
                                nc.scalar.activation(
                                    Bt["ktz"][0:64, mk, 0, S0:S], psB[0:64],
                                    AF.Identity, bias=bqk[0:64, m : m + 1],
                                    scale=1.0)
                            else:
                                nc.vector.tensor_scalar_add(
                                    Bt["ktz"][0:64, mk, 0, 0:S0], psA[0:64],
                                    bqk[0:64, m : m + 1])
                                nc.vector.tensor_scalar_add(
                                    Bt["ktz"][0:64, mk, 0, S0:S], psB[0:64],
                                    bqk[0:64, m : m + 1])
                            nc.vector.tensor_scalar_add(
                                Bt["ktz"][64:128, mk, 1, 0:S0], psA[64:128],
                                bqk[64:128, m : m + 1])
                            nc.vector.tensor_scalar_add(
                                Bt["ktz"][64:128, mk, 1, S0:S], psB[64:128],
                                bqk[64:128, m : m + 1])
                    yield

            def v_tile_steps(b, tt, alt=False):
                Bt = bat[b]
                tsz = min(128, S - tt * 128)
                t0 = tt * 128
                if alt:
                    ps = pcx.tile([128, D], F32, tag="cx", name=f"vP{b}_{tt}")
                    psA, psB = ps[:, 0:512], ps[:, 512:D]
                else:
                    psA = ppj.tile([128, 512], F32, tag="pj", name=f"vA{b}_{tt}")
                    psB = ppj.tile([128, 256], F32, tag="pj", name=f"vB{b}_{tt}")
                for dc in range(DC):
                    st, sp = dc == 0, dc == DC - 1
                    nc.tensor.matmul(
                        psA[:tsz], lhsT=Bt["xt"][:, dc, t0 : t0 + tsz],
                        rhs=wv[:, dc, 0:512], start=st, stop=sp)
                    nc.tensor.matmul(
                        psB[:tsz], lhsT=Bt["xt"][:, dc, t0 : t0 + tsz],
                        rhs=wv[:, dc, 512:D], start=st, stop=sp)
                    if sp:
                        nc.vector.tensor_add(
                            Bt["v"][:tsz, tt, 0:512], psA[:tsz], bvbc[:tsz, 0:512])
                        nc.vector.tensor_add(
                            Bt["v"][:tsz, tt, 512:D], psB[:tsz], bvbc[:tsz, 512:D])
                    yield

            def o_tile_steps(b, tt, alt=False):
                Bt = bat[b]
                tsz = min(128, S - tt * 128)
                t0 = tt * 128
                if alt:
                    ps = pcx.tile([128, D], F32, tag="cx", name=f"oP{b}_{tt}")
                    psA, psB = ps[:, 0:512], ps[:, 512:D]
                else:
                    psA = ppj.tile([128, 512], F32, tag="pj", name=f"oA{b}_{tt}")
                    psB = ppj.tile([128, 256], F32, tag="pj", name=f"oB{b}_{tt}")
                for fc in range(FC):
                    st, sp = fc == 0, fc == FC - 1
                    nc.tensor.matmul(
                        psA[:tsz], lhsT=Bt["ctxT"][:, fc, t0 : t0 + tsz],
                        rhs=wo[:, fc, 0:512], start=st, stop=sp)
                    nc.tensor.matmul(
                        psB[:tsz], lhsT=Bt["ctxT"][:, fc, t0 : t0 + tsz],
                        rhs=wo[:, fc, 512:D], start=st, stop=sp)
                    if sp:
                        ot = opool.tile([128, D], F32, tag="ot", name=f"ot{b}_{tt}")
                        nc.vector.tensor_add(
                            ot[:tsz, 0:512], psA[:tsz], bobc[:tsz, 0:512])
                        nc.vector.tensor_add(
                            ot[:tsz, 512:D], psB[:tsz], bobc[:tsz, 512:D])
                        nc.sync.dma_start(
                            out=out_d[b, t0 : t0 + tsz, :], in_=ot[:tsz])
                    yield

            def gen_fill(b):
                """Filler steps emitted during attention(b): output
                projection of b-1 and Q/K/V projections of b+1, chunk-
                sequential (the pj psum pool holds one chunk's two pieces)."""
                gens = []
                qgens = []
                if b + 1 < bc:
                    start_batch(b + 1)
                    # pair Q and K chunks so head h's operands finish early
                    order = []
                    for i in range(FC):
                        order += [i, FC + i]
                    qgens = [qk_chunk_steps(b + 1, m) for m in order]
                # For the last batch, hold back the final O tile: its steps
                # pad the epilogue's ctx pair, which otherwise stalls on the
                # very last exps.
                ott = TT - 1 if b == bc - 1 else TT
                ogens = [o_tile_steps(b - 1, tt) for tt in range(ott)] if b >= 1 else []
                # interleave at chunk granularity: q, o, q, o, ... then v
                qi, oi = iter(qgens), iter(ogens)
                while True:
                    qn = next(qi, None)
                    on = next(oi, None)
                    if qn is None and on is None:
                        break
                    if qn is not None:
                        gens.append(qn)
                    if on is not None:
                        gens.append(on)
                if b + 1 < bc:
                    gens += [v_tile_steps(b + 1, tt) for tt in range(TT)]
                for g in gens:
                    yield from g

            # ---- attention pieces
            def sc_step(b, h, kc):
                Bt = bat[b]
                m, hh = h // 2, h % 2
                ksz = min(128, S - kc * 128)
                k0 = kc * 128
                ps = psc.tile([128, S], F32, tag="sc", name=f"sc{b}_{h}_{kc}")
                nc.tensor.matmul(
                    ps[:ksz, 0:S0], lhsT=Bt["ktz"][:, m, hh, k0 : k0 + ksz],
                    rhs=Bt["qk"][:, m, 0:S0], start=True, stop=True)
                nc.tensor.matmul(
                    ps[:ksz, S0:S], lhsT=Bt["ktz"][:, m, hh, k0 : k0 + ksz],
                    rhs=Bt["qk"][:, m, S0:S], start=True, stop=True)
                nc.scalar.activation(
                    Bt["probs"][h][:ksz, kc, :], ps[:ksz, 0:S], AF.Exp,
                    accum_out=Bt["den"][h][:ksz, kc : kc + 1])

            def vs_prep(b, h):
                """1/den folded into this head's V columns, zero-padded to a
                full 128-col stationary operand (sub-128 tiling modes drop
                the PE out of its fast clock, so ctx uses full-array
                matmuls with the other head's half zeroed). The pad halves
                persist across pool reuse: 12 heads % 4 bufs = 0, so each
                buffer always carries the same head parity."""
                Bt = bat[b]
                rd = rdpool.tile([128, TT], F32, tag="rd", name=f"rd{b}_{h}")
                nc.vector.reciprocal(rd, Bt["den"][h])
                po = (h % 2) * 64
                vs = vspool.tile([128, TT, 128], BF16, tag="vs", name=f"vs{b}_{h}")
                if b == 0 and h < 4:
                    nc.vector.memset(vs[:, :, 64 - po : 128 - po], 0.0)
                for kc in range(TT):
                    ksz = min(128, S - kc * 128)
                    nc.vector.tensor_scalar_mul(
                        vs[:ksz, kc, po : po + DH],
                        Bt["v"][:ksz, kc, h * DH : (h + 1) * DH],
                        rd[:ksz, kc : kc + 1])
                Bt["vs"][h] = vs

            def ctx_pair(b, h0, fill=None):
                """ctx for heads (h0, h0+1): both accumulate into one
                [128, S] PSUM tile (head h0 owns partitions 0:64, h0+1 owns
                64:128 via the zero-padded halves of vs). At a batch
                boundary head h0+1's probs are still draining from ScalarE,
                so the caller passes `fill` to pad between the halves."""
                Bt = bat[b]
                ps = pcx.tile([128, S], F32, tag="cx", name=f"cx{b}_{h0}")
                for hh in (h0, h0 + 1):
                    for kc in range(TT):
                        ksz = min(128, S - kc * 128)
                        st = hh == h0 and kc == 0
                        sp = hh == h0 + 1 and kc == TT - 1
                        nc.tensor.matmul(
                            ps[:, 0:S0], lhsT=Bt["vs"][hh][:ksz, kc, :],
                            rhs=Bt["probs"][hh][:ksz, kc, 0:S0],
                            start=st, stop=sp)
                        nc.tensor.matmul(
                            ps[:, S0:S], lhsT=Bt["vs"][hh][:ksz, kc, :],
                            rhs=Bt["probs"][hh][:ksz, kc, S0:S],
                            start=st, stop=sp)
                    if hh == h0 and fill is not None:
                        emit_fill(fill, 3)
                nc.vector.tensor_copy(Bt["ctxT"][:, h0 // 2, :], ps[:, 0:S])
                # pair's probs/vs no longer needed; let pools rotate
                for hh in (h0, h0 + 1):
                    Bt["probs"].pop(hh, None)
                    Bt["vs"].pop(hh, None)

            # ---- the schedule
            def emit_fill(fill, n):
                for _ in range(n):
                    try:
                        next(fill)
                    except StopIteration:
                        return

            def slot(b, h, fill):
                Bt = bat[b]
                if h >= 1:
                    vs_prep(b, h - 1)
                probs = ppool.tile([128, TT, S], BF16, tag="probs",
                                   name=f"pr{b}_{h}")
                den = dpool.tile([128, TT], F32, tag="den", name=f"dn{b}_{h}")
                nc.vector.memset(den, 1.0)
                Bt["probs"][h] = probs
                Bt["den"][h] = den
                # paced emission: scores tiles gated by the exp chain two
                # tiles back (scores psum pool bufs=2); fillers keep the PE
                # fed while ScalarE drains.
                if h == 0:
                    # batch boundary: ScalarE is draining the previous
                    # batch's trailing exps. Issue next-batch DMAs + filler
                    # and the carry-over ctx pair BEFORE touching the scores
                    # psum pool so the PE never waits on that drain.
                    emit_fill(fill, 1)
                    if b >= 1:
                        ctx_pair(b - 1, 10, fill)
                    emit_fill(fill, 1)
                    sc_step(b, h, 0)
                    emit_fill(fill, 1); sc_step(b, h, 1)
                    emit_fill(fill, 1); sc_step(b, h, 2)
                    emit_fill(fill, 1); sc_step(b, h, 3)
                    emit_fill(fill, 1); sc_step(b, h, 4)
                    emit_fill(fill, 1)
                elif h == 1:
                    # the slot after a batch boundary inherits ScalarE
                    # backlog: front-load the fillers
                    emit_fill(fill, 2); sc_step(b, h, 0)
                    emit_fill(fill, 3); sc_step(b, h, 1)
                    emit_fill(fill, 3); sc_step(b, h, 2)
                    emit_fill(fill, 3); sc_step(b, h, 3)
                    emit_fill(fill, 3); sc_step(b, h, 4)
                    emit_fill(fill, 3)
                elif h % 2 == 1:
                    sc_step(b, h, 0)
                    emit_fill(fill, 2); sc_step(b, h, 1)
                    emit_fill(fill, 2); sc_step(b, h, 2)
                    emit_fill(fill, 3); sc_step(b, h, 3)
                    emit_fill(fill, 3); sc_step(b, h, 4)
                    emit_fill(fill, 5)
                else:
                    sc_step(b, h, 0)
                    emit_fill(fill, 1); sc_step(b, h, 1)
                    emit_fill(fill, 1); sc_step(b, h, 2)
                    ctx_pair(b, h - 2)
                    sc_step(b, h, 3)
                    emit_fill(fill, 2); sc_step(b, h, 4)
                    emit_fill(fill, 2)

            # prologue: batch 0 projections, nothing to overlap with.
            # DMA order: first QK weight chunks + x(0) (the first matmuls'
            # operands), then everything needed later. wqk is split so the
            # first chunks land fast without paying per-DMA setup 12 times.
            nc.sync.dma_start(out=bqk, in_=bqk_d[:])
            nc.sync.dma_start(out=wqk[:, 0], in_=wqk_d[:, 0])
            nc.sync.dma_start(out=wqk[:, FC], in_=wqk_d[:, FC])
            start_batch(0)
            nc.sync.dma_start(out=wqk[:, 1:FC], in_=wqk_d[:, 1:FC])
            nc.sync.dma_start(out=wqk[:, FC + 1 :], in_=wqk_d[:, FC + 1 :])
            nc.sync.dma_start(out=bvbc, in_=bvbc_d[:])
            nc.sync.dma_start(out=wv, in_=wv_d[:])
            nc.sync.dma_start(out=wo, in_=wo_d[:])
            nc.sync.dma_start(out=bobc, in_=bobc_d[:])
            order0 = []
            for i in range(FC):
                order0 += [i, FC + i]
            for idx, m in enumerate(order0):
                for _ in qk_chunk_steps(0, m, alt=(idx % 2 == 0)):
                    pass
            for tt in range(TT):
                for _ in v_tile_steps(0, tt, alt=(tt % 2 == 0)):
                    pass

            def throttled(gen, k):
                """Yield k times per real filler step so a short filler
                stream spreads over the whole batch (last batch has only
                the previous batch's output projection to offer)."""
                while True:
                    try:
                        next(gen)
                    except StopIteration:
                        return
                    for _ in range(k - 1):
                        yield
                    yield

            # main loop
            for b in range(bc):
                fill = gen_fill(b)
                if b == bc - 1:
                    fill = throttled(fill, 4)
                for h in range(H):
                    slot(b, h, fill)
                vs_prep(b, H - 1)
                # drain any remaining fillers at the batch boundary
                emit_fill(fill, 10**6)

            # epilogue: last pair (padded by the held-back O(bc-2) tile) +
            # output projection of the last batch
            heldback = o_tile_steps(bc - 2, TT - 1)
            ctx_pair(bc - 1, 10, heldback)
            emit_fill(heldback, 10**6)
            for tt in range(TT):
                for _ in o_tile_steps(bc - 1, tt, alt=(tt % 2 == 1)):
                    pass

    return nc


# ---------------------------------------------------------------- host prep
def _prep_shared(Wq, bq, Wk, bk, Wv, bv, Wo, bo):
    """Build the per-core-identical weight operands."""
    scale = np.float32(1.0 / np.sqrt(DH))
    wqf = (Wq.astype(np.float32) * scale).transpose(1, 0, 2).reshape(D, D)
    wkf = Wk.astype(np.float32).transpose(1, 0, 2).reshape(D, D)
    wvf = Wv.astype(np.float32).transpose(1, 0, 2).reshape(D, D)

    def chunk4(wf):  # [d, f] -> [di, m, dc, fi]
        return wf.reshape(DC, 128, FC, 128).transpose(1, 2, 0, 3)

    wqk = np.concatenate([chunk4(wqf), chunk4(wkf)], axis=1)  # [128, 12, 6, 128]
    wv3 = wvf.reshape(DC, 128, D).transpose(1, 0, 2)          # [128, 6, 768]
    wo3 = Wo.astype(np.float32).reshape(FC, 128, D).transpose(1, 0, 2)

    bqf = (bq.astype(np.float32) * scale).reshape(D)
    bkf = bk.astype(np.float32).reshape(D)
    bqk = np.concatenate(
        [bqf.reshape(FC, 128), bkf.reshape(FC, 128)], axis=0
    ).T.copy()                                                # [128, 12]
    bvbc = np.broadcast_to(bv.astype(np.float32).reshape(D), (128, D)).copy()
    bobc = np.broadcast_to(bo.astype(np.float32).reshape(D), (128, D)).copy()

    return {
        "wqk": np.ascontiguousarray(wqk).astype(nbf),
        "wv": np.ascontiguousarray(wv3).astype(nbf),
        "wo": np.ascontiguousarray(wo3).astype(nbf),
        "bqk": np.ascontiguousarray(bqk),
        "bvbc": bvbc,
        "bobc": bobc,
    }


_NC_CACHE = {}


def kernel(x, Wq, bq, Wk, bk, Wv, bv, Wo, bo):
    x = np.asarray(x, dtype=np.float32)
    shared = _prep_shared(
        np.asarray(Wq), np.asarray(bq), np.asarray(Wk), np.asarray(bk),
        np.asarray(Wv), np.asarray(bv), np.asarray(Wo), np.asarray(bo))

    in_maps = []
    for c in range(NCORES):
        xc = x[c * BC : (c + 1) * BC]                    # [BC, S, D]
        xt = xc.transpose(2, 0, 1)                       # [D, BC, S]
        xt = xt.reshape(DC, 128, BC, S).astype(nbf)
        m = dict(shared)
        m["xt"] = np.ascontiguousarray(xt)
        in_maps.append(m)

    if "nc" not in _NC_CACHE:
        _NC_CACHE["nc"] = build_bass()
    nc = _NC_CACHE["nc"]

    res = run_bass_kernel_spmd(nc, in_maps, core_ids=list(range(NCORES)))
    out = np.concatenate([res.results[c]["out"] for c in range(NCORES)], axis=0)
    return out.astype(np.float32)


if __name__ == "__main__":
    rng = np.random.default_rng(0)
    ins = {
        "x": rng.standard_normal((B, S, D), dtype=np.float32),
        "Wq": rng.standard_normal((H, D, DH), dtype=np.float32) * 0.02,
        "bq": np.zeros((H, DH), np.float32),
        "Wk": rng.standard_normal((H, D, DH), dtype=np.float32) * 0.02,
        "bk": np.zeros((H, DH), np.float32),
        "Wv": rng.standard_normal((H, D, DH), dtype=np.float32) * 0.02,
        "bv": np.zeros((H, DH), np.float32),
        "Wo": rng.standard_normal((D, D), dtype=np.float32) * 0.02,
        "bo": np.zeros((D,), np.float32),
    }
    o = kernel(**ins)
    print("out", o.shape, o.dtype, float(np.abs(o).max()))
